# revision 47
# baseline (speedup 1.0000x reference)
"""Trainium2 Bass kernel for GQA attention (B=2, L=2048, D=2048, H=16, KV=8, HD=128).

Sharding: 2-way data-parallel over batch x 4-way tensor-parallel over heads
(KV-head groups intact). Each core handles one batch and 4 query heads
(2 KV heads): QKV projection + RoPE + RMSNorm + flash-style attention +
a PARTIAL output projection over its 4 heads' rows of wo (row-sharded wo).
The host gather SUMS the 4 partial [D, L] outputs per batch — no on-device
collective. Avoiding collective_compute keeps the PE at full clock.

All heavy matmuls are fp16 with fp32 PSUM accumulation. rstd and softmax
denominators use the Ln/Exp one-table trick and PE ones/broadcast matmuls.

v2 schedule: full-width RoPE ops (host-precomputed [cos;cos] and [-sin;+sin]
tables), paired exp tiles for half-count softmax accumulation adds, and
q-projection half-chains woven into the early attention pairs so the PE
stays fed while the ACT exp stream is the per-pair wall.
"""
import numpy as np

B, L, D = 2, 2048, 2048
H, KV, HD = 16, 8, 128
NCORES = 8
HPC = 4              # query heads per core
KPC = 2              # kv heads per core
EPS = 1e-5
ROPE_BASE = 10000.0
SCALE = HD ** -0.5

TT = 512             # token tile (free dim)
NTT = L // TT        # 4 token tiles
NDC = D // 128       # 16 contraction chunks in qkv proj
NFC = 8              # 4 q + 2 k + 2 v column chunks of 128
QKV_COLS = NFC * 128

_CACHE = {}


def _rope_tables():
    """[128, L] cos2=[cos;cos] and sin2=[-sin;+sin] LUTs, matching the jax
    reference numerics (f32, cpu)."""
    import jax
    import jax.numpy as jnp

    cpu = jax.devices("cpu")[0]
    with jax.default_device(cpu):
        base = ROPE_BASE * 1.0 ** (HD / (HD - 2))
        freqs = base ** (jnp.arange(0, HD, 2, dtype=jnp.float32) / HD)   # [64]
        pos = jnp.arange(L, dtype=jnp.float32)                           # [L]
        angles = pos[:, None] * freqs[None, :]                           # [L, 64]
        cos = np.asarray(jnp.cos(angles), dtype=np.float32).T            # [64, L]
        sin = np.asarray(jnp.sin(angles), dtype=np.float32).T
    cos2 = np.concatenate([cos, cos], axis=0)     # [128, L]
    sin2 = np.concatenate([sin, -sin], axis=0)    # [128, L]
    return np.ascontiguousarray(cos2), np.ascontiguousarray(sin2)


def _build_nc():
    import concourse.bass as bass
    import concourse.tile as tile
    import concourse.mybir as mybir
    from concourse import bacc
    from concourse.masks import make_identity
    from contextlib import ExitStack

    f32 = mybir.dt.float32
    f16 = mybir.dt.float16
    Exp = mybir.ActivationFunctionType.Exp
    Ln = mybir.ActivationFunctionType.Ln
    Copy = mybir.ActivationFunctionType.Copy
    mult = mybir.AluOpType.mult
    add = mybir.AluOpType.add

    from concourse import bacc as _bacc_mod

    if not getattr(_bacc_mod, "_act_table_patch", False):
        _orig_get = _bacc_mod.get_activation_tables

        def _patched_get(arch):
            t = _orig_get(arch)
            exp = mybir.ActivationFunctionType.Exp
            ln = mybir.ActivationFunctionType.Ln
            for name, funcs in t.items():
                if name != "natural_log_exp_and_others":
                    funcs.discard(exp)
                    funcs.discard(ln)
            return t

        _bacc_mod.get_activation_tables = _patched_get
        _bacc_mod._act_table_patch = True

    nc = bacc.Bacc(num_devices=NCORES)

    # per-core inputs (host pre-sliced)
    xT = nc.dram_tensor("xT", [D, L], f16, kind="ExternalInput")
    wqkv = nc.dram_tensor("wqkv", [D, QKV_COLS], f16, kind="ExternalInput")
    # wo rows for this core's 4 heads, tiled [128, h, oc, 128]
    woT = nc.dram_tensor("woT", [128, HPC * NDC * 128], f16, kind="ExternalInput")
    lcos = nc.dram_tensor("lcos", [128, L], f16, kind="ExternalInput")
    lsin = nc.dram_tensor("lsin", [128, L], f16, kind="ExternalInput")
    qn = nc.dram_tensor("qn", [HD, 1], f32, kind="ExternalInput")
    kn = nc.dram_tensor("kn", [HD, 1], f32, kind="ExternalInput")
    # partial output: yT[oc, tok] = sum over this core's heads
    yT = nc.dram_tensor("yT", [D, L], f16, kind="ExternalOutput")

    with tile.TileContext(nc) as tc, ExitStack() as ctx, nc.allow_low_precision(
        reason="f16 storage; all matmul accumulation is fp32 PSUM"
    ):
        consts = ctx.enter_context(tc.tile_pool(name="consts", bufs=1))
        qkvp = ctx.enter_context(tc.tile_pool(name="qkvp", bufs=1))
        ropep = ctx.enter_context(tc.tile_pool(name="ropep", bufs=2))
        statp = ctx.enter_context(tc.tile_pool(name="statp", bufs=2))
        sap = ctx.enter_context(tc.tile_pool(name="sap", bufs=4))
        expp = ctx.enter_context(tc.tile_pool(name="expp", bufs=4))
        attp = ctx.enter_context(tc.tile_pool(name="attp", bufs=2))
        bcp = ctx.enter_context(tc.tile_pool(name="bcp", bufs=2))
        yp = ctx.enter_context(tc.tile_pool(name="yp", bufs=4))

        pbig = ctx.enter_context(tc.tile_pool(name="pbig", bufs=2, space="PSUM"))
        patt = ctx.enter_context(tc.tile_pool(name="patt", bufs=2, space="PSUM"))
        pscore = ctx.enter_context(tc.tile_pool(name="pscore", bufs=4, space="PSUM"))

        # ---- resident tensors. DMA priority order, split small so the
        # 16 queues parallelize and the first wave (kv cols x tt0/tt1)
        # streams right behind the DMA:
        #   kv w-cols + x tt0/tt1 per dc -> x tt2/tt3 -> q w-cols ----
        # Each DMA queue delivers ~15 GB/s (descriptor processing scales with
        # bytes); 16 queues ~ 240 GB/s aggregate. The first wave consumes one
        # dc (8 MMs) per 1.73us, i.e. ~380KB/dc of supply budget — so phase 1
        # carries ONLY w-kv cols + x tt0/tt1 per dc (384KB), partition-halved
        # so per-dc latency is ~8.6us with a ~1.6us/dc pipelined cadence.
        x_sb = consts.tile([128, NDC, L], f16)
        _xr = xT.ap().rearrange("(dc p) t -> p dc t", p=128)
        w_sb = consts.tile([128, NDC, QKV_COLS], f16)
        _wr = wqkv.ap().rearrange("(dc p) f -> p dc f", p=128)

        # dma_start issue costs ~600ns on the issuing sequencer; rotate the
        # issuing engine so 4 queues' worth issue in parallel at startup.
        _dma_engines = [nc.sync, nc.gpsimd, nc.scalar]
        _di = [0]

        def dma(out, in_):
            e = _dma_engines[_di[0] % 3]
            _di[0] += 1
            e.dma_start(out=out, in_=in_)

        for dc in range(NDC):
            nsp = 4 if dc < 2 else 2
            for s in range(nsp):
                p0, p1 = s * 128 // nsp, (s + 1) * 128 // nsp
                dma(w_sb[p0:p1, dc:dc + 1, 512:1024],
                    _wr[p0:p1, dc:dc + 1, 512:1024])
            for s in range(nsp):
                p0, p1 = s * 128 // nsp, (s + 1) * 128 // nsp
                dma(x_sb[p0:p1, dc:dc + 1, 0:2 * TT],
                    _xr[p0:p1, dc:dc + 1, 0:2 * TT])
        # phase 2: x tt2/tt3 (wave2 feed), then q weight cols (proj_tt(0) feed)
        for dc in range(NDC):
            for s in range(2):
                p0, p1 = s * 64, (s + 1) * 64
                dma(x_sb[p0:p1, dc:dc + 1, 2 * TT:L],
                    _xr[p0:p1, dc:dc + 1, 2 * TT:L])
        for dc in range(NDC):
            for s in range(2):
                p0, p1 = s * 64, (s + 1) * 64
                dma(w_sb[p0:p1, dc:dc + 1, 0:512],
                    _wr[p0:p1, dc:dc + 1, 0:512])
        # ---- constants ----
        ones_f = consts.tile([128, 1], f32)
        nc.vector.memset(ones_f, 1.0)
        ones = consts.tile([128, 1], f16)
        nc.vector.tensor_copy(out=ones, in_=ones_f)
        ident = consts.tile([128, 128], f16)
        make_identity(nc, ident)
        eps_t = consts.tile([1, 1], f32)
        nc.vector.memset(eps_t, EPS)
        neg8 = consts.tile([128, 1], f32)
        nc.vector.memset(neg8, -8.0)
        cos_sb = consts.tile([128, L], f16)
        nc.gpsimd.dma_start(out=cos_sb, in_=lcos[:, :])
        sin_sb = consts.tile([128, L], f16)
        nc.gpsimd.dma_start(out=sin_sb, in_=lsin[:, :])
        qn_sb = consts.tile([HD, 1], f32)
        nc.gpsimd.dma_start(out=qn_sb, in_=qn[:, :])
        kn_sb = consts.tile([HD, 1], f32)
        nc.gpsimd.dma_start(out=kn_sb, in_=kn[:, :])
        # wo tiles [128, h, oc, 128] (needed only at out-projection)
        wo_sb = consts.tile([128, HPC, NDC, 128], f16)
        _wor = woT.ap().rearrange("p (h oc m) -> p h oc m", h=HPC, oc=NDC)
        nc.sync.dma_start(out=wo_sb[:, 0:2, :, :], in_=_wor[:, 0:2, :, :])
        nc.sync.dma_start(out=wo_sb[:, 2:4, :, :], in_=_wor[:, 2:4, :, :])

        # activations
        qh_t = [qkvp.tile([128, L], f16, name=f"qh{h}") for h in range(HPC)]
        kh_t = [qkvp.tile([128, L], f16, name=f"kh{g}") for g in range(KPC)]
        v_t = [qkvp.tile([128, L // 128, HD], f16, name=f"v{g}") for g in range(KPC)]
        a_t = [qkvp.tile([128, L], f16, name=f"a{h}") for h in range(HPC)]

        def proj_post(tt, fc, rsrc, act_swap=False, pss=None):
            """RoPE + RMSNorm (q/k) or transpose (v) for a staged projection tile.

            RoPE with full-width ops: cos_sb = [cos;cos], sin_sb = [-sin;+sin];
            roped = rsrc*cos2 + swap(rsrc)*sin2."""
            pos0 = tt * TT
            if fc < 6:
                # RoPE preserves per-token sum of squares, so the rstd chain
                # runs off the pre-rope tile, in parallel with the rotation.
                sq = ropep.tile([128, TT], f16, tag="tb", name="sq")
                nc.vector.tensor_tensor(out=sq, in0=rsrc, in1=rsrc, op=mult)
                if pss is None:
                    pt = pscore.tile([1, TT], f32, tag="ps", name="pt")
                else:
                    pt = pscore.tile([1, TT], f32, tag="ps", name="ptsm")
                nc.tensor.matmul(pt, ones, sq, start=True, stop=True)
                lsrc = pt
                lnt = statp.tile([1, TT], f32, tag="statf")
                nc.scalar.activation(
                    out=lnt, in_=lsrc, func=Ln, bias=eps_t, scale=1.0 / HD
                )
                srd = statp.tile([1, TT], f16, tag="stath")
                nc.scalar.activation(out=srd, in_=lnt, func=Exp, scale=-0.5)
                pb = bcp.tile([128, TT], f16, tag="bc")
                nc.gpsimd.partition_broadcast(out_ap=pb, in_ap=srd)
                # rotation: ta = rsrc*[cos;cos]; tbs = rsrc*[+sin;-sin];
                # tb2 = swap(tbs) halves; roped = ta + tb2 (in place).
                ta = ropep.tile([128, TT], f16, tag="ta")
                nc.vector.tensor_tensor(
                    out=ta, in0=rsrc, in1=cos_sb[:, pos0:pos0 + TT], op=mult
                )
                tbs = ropep.tile([128, TT], f16, tag="tb", name="tbs")
                nc.vector.tensor_tensor(
                    out=tbs, in0=rsrc, in1=sin_sb[:, pos0:pos0 + TT], op=mult
                )
                tb2 = ropep.tile([128, TT], f16, tag="tb2")
                if act_swap:
                    # offload the half swaps to ACT in phases where it idles
                    nc.scalar.activation(
                        out=tb2[0:64, :], in_=tbs[64:128, :], func=Copy
                    )
                    nc.scalar.activation(
                        out=tb2[64:128, :], in_=tbs[0:64, :], func=Copy
                    )
                else:
                    nc.vector.tensor_copy(out=tb2[0:64, :], in_=tbs[64:128, :])
                    nc.vector.tensor_copy(out=tb2[64:128, :], in_=tbs[0:64, :])
                roped = ta
                nc.vector.tensor_tensor(out=roped, in0=ta, in1=tb2, op=add)
                w_head = qn_sb if fc < 4 else kn_sb
                if fc < 4:
                    dst = qh_t[fc][:, pos0:pos0 + TT]
                else:
                    dst = kh_t[fc - 4][:, pos0:pos0 + TT]
                nc.vector.scalar_tensor_tensor(
                    out=dst, in0=roped, scalar=w_head, in1=pb,
                    op0=mult, op1=mult,
                )
            else:
                g = fc - 6
                for i in range(TT // 128):
                    pt = pscore.tile([128, 128], f16, tag="ps")
                    nc.tensor.transpose(pt, rsrc[:, i * 128:(i + 1) * 128], ident)
                    nc.vector.tensor_copy(out=v_t[g][:, tt * 4 + i, :], in_=pt)

        def proj_chain(tt, fc):
            """Emit the 16-chunk projection chain for (tt, fc); return staged copy."""
            pos0 = tt * TT
            pp = pbig.tile([128, TT], f32, tag="pp", name=f"pp{fc}")
            for dc in range(NDC):
                nc.tensor.matmul(
                    pp,
                    w_sb[:, dc, fc * 128:(fc + 1) * 128],
                    x_sb[:, dc, pos0:pos0 + TT],
                    start=(dc == 0),
                    stop=(dc == NDC - 1),
                )
            rsrc = ropep.tile([128, TT], f16, tag="rsrc", bufs=4)
            nc.vector.tensor_copy(out=rsrc, in_=pp)
            return rsrc

        def proj_units(tt, fc):
            """Split one (tt, fc) projection chain into weave units:
            [8 MMs] [8 MMs + drain] [post]."""
            pos0 = tt * TT
            st = {}

            def ua():
                st["pp"] = pbig.tile([128, TT], f32, tag="pp", name=f"pw{fc}{tt}")
                for dc in range(8):
                    nc.tensor.matmul(
                        st["pp"],
                        w_sb[:, dc, fc * 128:(fc + 1) * 128],
                        x_sb[:, dc, pos0:pos0 + TT],
                        start=(dc == 0),
                        stop=False,
                    )

            def ub():
                for dc in range(8, NDC):
                    nc.tensor.matmul(
                        st["pp"],
                        w_sb[:, dc, fc * 128:(fc + 1) * 128],
                        x_sb[:, dc, pos0:pos0 + TT],
                        start=False,
                        stop=(dc == NDC - 1),
                    )
                rsrc = ropep.tile([128, TT], f16, tag="rsrc", bufs=4)
                nc.vector.tensor_copy(out=rsrc, in_=st["pp"])
                st["rsrc"] = rsrc

            def uc():
                # ssq via psmall: don't steal a score bank mid-attention
                proj_post(tt, fc, st["rsrc"])

            return [ua, ub, uc]

        def proj_wave(pairs, stage_dsts):
            """8 projection chains at once (one PSUM bank each, borrowed across
            pools) so the PE keeps pace with the initial per-dc x DMA stream.

            Results are staged into stage_dsts (idle a_t slices — unused until
            the attention finalizers, long after the posts consume them), so
            no staging-ring pressure. Returns deferred post closures."""
            slots = [
                pbig.tile([128, TT], f32, tag="pp", name="wv0"),
                pbig.tile([128, TT], f32, tag="pp", name="wv1"),
                patt.tile([128, TT], f32, tag="po", name="wv2"),
                patt.tile([128, TT], f32, tag="po", name="wv3"),
                pscore.tile([128, TT], f32, tag="ps", name="wv4"),
                pscore.tile([128, TT], f32, tag="ps", name="wv5"),
                pscore.tile([128, TT], f32, tag="ps", name="wv6"),
                pscore.tile([128, TT], f32, tag="ps", name="wv7"),
            ]
            assert len(pairs) == 8
            for dc in range(NDC):
                for i, (fc, tt) in enumerate(pairs):
                    pos0 = tt * TT
                    nc.tensor.matmul(
                        slots[i],
                        w_sb[:, dc, fc * 128:(fc + 1) * 128],
                        x_sb[:, dc, pos0:pos0 + TT],
                        start=(dc == 0),
                        stop=(dc == NDC - 1),
                    )
            staged = []
            for i, (fc, tt) in enumerate(pairs):
                dst = stage_dsts[i]
                nc.vector.tensor_copy(out=dst, in_=slots[i])
                staged.append((tt, fc, dst))
            return staged

        def proj_tt(tt, fcs, sprinkle=()):
            """Projection for one token tile: chains run two fc ahead of
            postprocessing; deferred wave posts sprinkle between chains."""
            sprinkle = list(sprinkle)
            pend = []
            for fc in fcs:
                rsrc = proj_chain(tt, fc)
                if sprinkle:
                    sprinkle.pop(0)()
                if sprinkle:
                    sprinkle.pop(0)()
                pend.append((fc, rsrc))
                if len(pend) > 1:
                    wfc, wrs = pend.pop(0)
                    proj_post(tt, wfc, wrs)
            for u in sprinkle:
                u()
            for wfc, wrs in pend:
                proj_post(tt, wfc, wrs)

        def outproj_chain(tt, oc, drain_act=False):
            """One 128-col partial out-projection chain + drain."""
            pos0 = tt * TT
            py = pbig.tile([128, TT], f32, tag="pp", name=f"py{oc % 2}")
            for h in range(HPC):
                nc.tensor.matmul(
                    py, wo_sb[:, h, oc, :], a_t[h][:, pos0:pos0 + TT],
                    start=(h == 0), stop=(h == HPC - 1),
                )
            yt = yp.tile([128, TT], f16, tag="y")
            if drain_act:
                nc.scalar.activation(out=yt, in_=py, func=Copy, scale=1.0)
            else:
                nc.vector.tensor_copy(out=yt, in_=py)
            nc.sync.dma_start(
                out=yT[oc * 128:(oc + 1) * 128, pos0:pos0 + TT], in_=yt
            )

        def oc_unit(tt, oc, drain_act=False):
            return lambda: outproj_chain(tt, oc, drain_act=drain_act)

        def att_tiles(jobs, weave=(), fin_prev=None):
            """Interleaved attention for (h, tqt) query tiles (same kv group).

            weave: list of closures (out-projection chains / q-proj units)
            emitted between key chunks to keep the PE fed while the ACT exp
            stream drains. fin_prev: finish closure of the previous pair,
            emitted after the first key chunks so its DVE-latency-bound
            normalize chain hides under this pair's score stream. Returns
            this pair's finish closure.

            exp tiles are paired [128, 2, TT] so the softmax running-sum adds
            run at [128, 2*TT] width, halving the DVE instruction count."""
            NK = L // 128
            weave = list(weave)
            state = []
            for h, tqt in jobs:
                g = h // 2
                qs = qh_t[h][:, tqt * TT:(tqt + 1) * TT]
                po = patt.tile([128, TT], f32, tag="po", name=f"po{h}{tqt}")
                sacc = sap.tile([128, 2, TT], f16, tag="sacc", bufs=3,
                                name=f"sa{h}{tqt}")
                state.append((h, g, tqt, qs, po, sacc, {}))
            nweave = len(weave)
            for tk in range(NK):
                for h, g, tqt, qs, po, sacc, st in state:
                    ps = pscore.tile([128, TT], f32, tag="ps")
                    nc.tensor.matmul(
                        ps, kh_t[g][:, tk * 128:(tk + 1) * 128], qs,
                        start=True, stop=True,
                    )
                    if tk % 2 == 0:
                        st["et2"] = expp.tile(
                            [128, 2, TT], f16, tag="expt", name=f"et{h}{tk}"
                        )
                    et = st["et2"][:, tk % 2, :]
                    # -8 exponent shift (softmax-invariant) keeps the po/v
                    # accumulator magnitudes small enough for f16 staging
                    nc.scalar.activation(
                        out=et, in_=ps, func=Exp, bias=neg8, scale=SCALE
                    )
                if tk == 0 and fin_prev is not None:
                    fin_prev()
                    fin_prev = None
                for h, g, tqt, qs, po, sacc, st in state:
                    et = st["et2"][:, tk % 2, :]
                    nc.tensor.matmul(
                        po, v_t[g][:, tk, :], et,
                        start=(tk == 0), stop=(tk == NK - 1),
                    )
                    if tk % 2 == 1:
                        pair = st["et2"][:, :, :]
                        sview = sacc[:, :, :]
                        if tk == 1:
                            nc.vector.tensor_copy(out=sview, in_=pair)
                        elif tk % 8 == 7:
                            # offload 2 of 7 running-sum adds to idle GpSimd
                            nc.gpsimd.tensor_tensor(
                                out=sview, in0=sview, in1=pair, op=add
                            )
                        else:
                            nc.vector.tensor_tensor(
                                out=sview, in0=sview, in1=pair, op=add
                            )
                # weave filler chains uniformly across the key loop
                while weave and len(weave) > nweave * (NK - 1 - tk) // NK:
                    weave.pop(0)()

            def fin(tail_weave=()):
                pds = []
                for h, g, tqt, qs, po, sacc, st in state:
                    pd = pscore.tile([1, TT], f32, tag="ps", name="pd")
                    nc.tensor.matmul(
                        pd, ones, sacc[:, 0, :], start=True, stop=False
                    )
                    nc.tensor.matmul(
                        pd, ones, sacc[:, 1, :], start=False, stop=True
                    )
                    rdf = statp.tile([1, TT], f32, tag="statf")
                    nc.vector.reciprocal_approx_fast(out=rdf, in_=pd)
                    pds.append(rdf)
                for u in tail_weave:
                    u()
                # Batches of 2: po-releasing copies and broadcast issues first,
                # broadcast-dependent multiplies after, so the in-order DVE
                # queue doesn't stall on a GpSimd broadcast before freeing po.
                pairs2 = [(s[0], s[2], s[4], rdf) for s, rdf in zip(state, pds)]
                for j0 in range(0, len(pairs2), 2):
                    staged2 = []
                    for h, tqt, po, rdf in pairs2[j0:j0 + 2]:
                        rd = statp.tile([1, TT], f16, tag="stath")
                        nc.vector.tensor_copy(out=rd, in_=rdf)
                        pb = bcp.tile([128, TT], f16, tag="bc")
                        nc.gpsimd.partition_broadcast(out_ap=pb, in_ap=rd)
                        o_sb = attp.tile([128, TT], f16, tag="att")
                        nc.vector.tensor_copy(out=o_sb, in_=po)
                        staged2.append((h, tqt, pb, o_sb))
                    for h, tqt, pb, o_sb in staged2:
                        nc.vector.tensor_tensor(
                            out=a_t[h][:, tqt * TT:(tqt + 1) * TT],
                            in0=o_sb, in1=pb, op=mult,
                        )

            return fin

        def outproj_tt(tt, drain_act=False):
            for oc in range(NDC):
                outproj_chain(tt, oc, drain_act=drain_act and oc % 2 == 0)

        # ---- schedule ----
        # KV proj waves first (posts deferred into the next MM stream);
        # q proj tt0 dense; q proj tt1-3 woven into the early attention pairs
        # as half-chain units; the out-projection woven into later pairs so
        # the PE never starves while the ACT exp stream drains.
        def wave_dsts(a_lo, a_hi):
            return [a_t[a_lo][:, s * TT:(s + 1) * TT] for s in range(4)] + [
                a_t[a_hi][:, s * TT:(s + 1) * TT] for s in range(4)
            ]

        st1 = proj_wave(
            [(4, 0), (5, 0), (6, 0), (7, 0), (4, 1), (5, 1), (6, 1), (7, 1)],
            stage_dsts=wave_dsts(0, 1),
        )
        st2 = proj_wave(
            [(4, 2), (5, 2), (6, 2), (7, 2), (4, 3), (5, 3), (6, 3), (7, 3)],
            stage_dsts=wave_dsts(2, 3),
        )

        def wpost(item, **kw):
            tt, fc, r = item
            proj_post(tt, fc, r, **kw)

        # wave1 posts, v-transposes first (their data staged long before so
        # the PE never waits on fresh DVE); k-posts share a packed ssq tile
        for i in (2, 3, 6, 7):
            wpost(st1[i])
        for i in (0, 1, 4, 5):
            wpost(st1[i], act_swap=True)
        # proj_tt(0) by hand: wave2's v-posts + 2 k-posts sprinkle between
        # chains; the (4,3)/(5,3) k-posts and the q2/q3 posts defer into
        # pair A's weave (ACT slack there; their consumers come later).
        r0 = proj_chain(0, 0)
        wpost(st2[2])                     # v (6,2)
        wpost(st2[3])                     # v (7,2)
        r1 = proj_chain(0, 1)
        wpost(st2[6])                     # v (6,3)
        wpost(st2[7])                     # v (7,3)
        proj_post(0, 0, r0, act_swap=True)
        r2 = proj_chain(0, 2)
        wpost(st2[0], act_swap=True)   # k (4,2)
        wpost(st2[1], act_swap=True)   # k (5,2)
        proj_post(0, 1, r1, act_swap=True)
        r3 = proj_chain(0, 3)

        w_a = [lambda: wpost(st2[4]),            # k (4,3)
               lambda: wpost(st2[5]),            # k (5,3)
               lambda: proj_post(0, 2, r2),
               lambda: proj_post(0, 3, r3)]
        w_a += proj_units(1, 0) + proj_units(1, 1)
        fin = att_tiles([(0, 0), (1, 0)], weave=w_a)
        w_b = proj_units(1, 2) + proj_units(1, 3)
        fin = att_tiles([(2, 0), (3, 0)], weave=w_b, fin_prev=fin)
        w_c = proj_units(2, 0) + proj_units(2, 1)
        fin = att_tiles([(0, 1), (1, 1)], weave=w_c, fin_prev=fin)
        w_d = proj_units(2, 2) + proj_units(2, 3)
        fin = att_tiles([(2, 1), (3, 1)], weave=w_d, fin_prev=fin)
        w_e = proj_units(3, 0) + proj_units(3, 1) + [oc_unit(0, 0), oc_unit(0, 1)]
        fin = att_tiles([(0, 2), (1, 2)], weave=w_e, fin_prev=fin)
        w_f = proj_units(3, 2) + proj_units(3, 3) + [oc_unit(0, 2), oc_unit(0, 3)]
        fin = att_tiles([(2, 2), (3, 2)], weave=w_f, fin_prev=fin)
        w_g = [oc_unit(0, oc) for oc in range(4, 10)] + [oc_unit(1, oc) for oc in range(0, 2)]
        fin = att_tiles([(0, 3), (1, 3)], weave=w_g, fin_prev=fin)
        w_h = [oc_unit(0, oc) for oc in range(10, 16)] + [oc_unit(1, oc) for oc in range(2, 4)]
        fin = att_tiles([(2, 3), (3, 3)], weave=w_h, fin_prev=fin)
        fin(tail_weave=[oc_unit(1, oc) for oc in range(4, 8)])
        # dense tail: alternate drain engines so neither DVE nor ACT
        # serializes the chain pipeline
        tail = [(1, oc) for oc in range(8, 16)]
        tail += [(2, oc) for oc in range(NDC)]
        tail += [(3, oc) for oc in range(NDC)]
        for i, (tt, oc) in enumerate(tail):
            outproj_chain(tt, oc, drain_act=i % 2 == 0)

    nc.finalize()
    return nc


def kernel(x, wq, wk, wv, wo, qn_w, kn_w):
    from concourse.bass_utils import run_bass_kernel_spmd

    if "nc" not in _CACHE:
        _CACHE["nc"] = _build_nc()
    nc = _CACHE["nc"]

    x = np.asarray(x, dtype=np.float32)
    wq = np.asarray(wq, dtype=np.float32)
    wk = np.asarray(wk, dtype=np.float32)
    wv = np.asarray(wv, dtype=np.float32)
    wo = np.asarray(wo, dtype=np.float32)
    qn_w = np.asarray(qn_w, dtype=np.float32).reshape(HD, 1).copy()
    kn_w = np.asarray(kn_w, dtype=np.float32).reshape(HD, 1).copy()

    cos2, sin2 = _rope_tables()
    cos2 = cos2.astype(np.float16)
    sin2 = sin2.astype(np.float16)

    in_maps = []
    for c in range(NCORES):
        bc, hc4 = divmod(c, 4)
        xT_c = np.ascontiguousarray(x[bc].T.astype(np.float16))
        wqkv_c = np.ascontiguousarray(
            np.concatenate(
                [
                    wq[:, hc4 * HPC * HD:(hc4 + 1) * HPC * HD],
                    wk[:, hc4 * KPC * HD:(hc4 + 1) * KPC * HD],
                    wv[:, hc4 * KPC * HD:(hc4 + 1) * KPC * HD],
                ],
                axis=1,
            ).astype(np.float16)
        )
        # wo rows for this core's heads -> [128, h*oc*128]
        wo_sl = wo[hc4 * HPC * HD:(hc4 + 1) * HPC * HD, :].astype(np.float16)
        wo_t = np.ascontiguousarray(
            wo_sl.reshape(HPC, 128, NDC, 128).transpose(1, 0, 2, 3).reshape(
                128, HPC * NDC * 128
            )
        )
        in_maps.append(
            {
                "xT": xT_c,
                "wqkv": wqkv_c,
                "woT": wo_t,
                "lcos": cos2,
                "lsin": sin2,
                "qn": qn_w,
                "kn": kn_w,
            }
        )

    trace = bool(_CACHE.get("trace"))
    r = run_bass_kernel_spmd(
        nc, in_maps, core_ids=list(range(NCORES)), trace=trace
    )
    _CACHE["last_result"] = r

    y = np.empty((B, L, D), dtype=np.float32)
    for bc in range(B):
        acc = r.results[bc * 4]["yT"].astype(np.float32)
        for hc4 in range(1, 4):
            acc += r.results[bc * 4 + hc4]["yT"]
        y[bc] = acc.T
    return y


# revision 49
# speedup vs baseline: 1.4675x; 1.4675x over previous
"""Trainium2 Bass kernel for GQA attention (B=2, L=2048, D=2048, H=16, KV=8, HD=128).

Sharding: 2-way data-parallel over batch x 4-way tensor-parallel over heads
(KV-head groups intact). Each core handles one batch and 4 query heads
(2 KV heads): QKV projection + RoPE + RMSNorm + flash-style attention +
a PARTIAL output projection over its 4 heads' rows of wo (row-sharded wo).
The host gather SUMS the 4 partial [D, L] outputs per batch — no on-device
collective. Avoiding collective_compute keeps the PE at full clock.

All heavy matmuls are fp16 with fp32 PSUM accumulation. rstd and softmax
denominators use the Ln/Exp one-table trick and PE ones/broadcast matmuls.

v2 schedule: full-width RoPE ops (host-precomputed [cos;cos] and [-sin;+sin]
tables), paired exp tiles for half-count softmax accumulation adds, and
q-projection half-chains woven into the early attention pairs so the PE
stays fed while the ACT exp stream is the per-pair wall.
"""
import numpy as np

B, L, D = 2, 2048, 2048
H, KV, HD = 16, 8, 128
NCORES = 8
HPC = 4              # query heads per core
KPC = 2              # kv heads per core
EPS = 1e-5
ROPE_BASE = 10000.0
SCALE = HD ** -0.5

TT = 512             # token tile (free dim)
NTT = L // TT        # 4 token tiles
NDC = D // 128       # 16 contraction chunks in qkv proj
NFC = 8              # 4 q + 2 k + 2 v column chunks of 128
QKV_COLS = NFC * 128

_CACHE = {}


def _rope_tables():
    """[128, L] cos2=[cos;cos] and sin2=[-sin;+sin] LUTs, matching the jax
    reference numerics (f32, cpu)."""
    import jax
    import jax.numpy as jnp

    cpu = jax.devices("cpu")[0]
    with jax.default_device(cpu):
        base = ROPE_BASE * 1.0 ** (HD / (HD - 2))
        freqs = base ** (jnp.arange(0, HD, 2, dtype=jnp.float32) / HD)   # [64]
        pos = jnp.arange(L, dtype=jnp.float32)                           # [L]
        angles = pos[:, None] * freqs[None, :]                           # [L, 64]
        cos = np.asarray(jnp.cos(angles), dtype=np.float32).T            # [64, L]
        sin = np.asarray(jnp.sin(angles), dtype=np.float32).T
    cos2 = np.concatenate([cos, cos], axis=0)     # [128, L]
    sin2 = np.concatenate([sin, -sin], axis=0)    # [128, L]
    return np.ascontiguousarray(cos2), np.ascontiguousarray(sin2)


def _build_nc():
    import concourse.bass as bass
    import concourse.tile as tile
    import concourse.mybir as mybir
    from concourse import bacc
    from concourse.masks import make_identity
    from contextlib import ExitStack

    f32 = mybir.dt.float32
    f16 = mybir.dt.float16
    Exp = mybir.ActivationFunctionType.Exp
    Ln = mybir.ActivationFunctionType.Ln
    Copy = mybir.ActivationFunctionType.Copy
    mult = mybir.AluOpType.mult
    add = mybir.AluOpType.add

    from concourse import bacc as _bacc_mod

    if not getattr(_bacc_mod, "_act_table_patch", False):
        _orig_get = _bacc_mod.get_activation_tables

        def _patched_get(arch):
            t = _orig_get(arch)
            exp = mybir.ActivationFunctionType.Exp
            ln = mybir.ActivationFunctionType.Ln
            for name, funcs in t.items():
                if name != "natural_log_exp_and_others":
                    funcs.discard(exp)
                    funcs.discard(ln)
            return t

        _bacc_mod.get_activation_tables = _patched_get
        _bacc_mod._act_table_patch = True

    nc = bacc.Bacc(num_devices=NCORES)

    # per-core inputs (host pre-sliced)
    xT = nc.dram_tensor("xT", [D, L], f16, kind="ExternalInput")
    wqkv = nc.dram_tensor("wqkv", [D, QKV_COLS], f16, kind="ExternalInput")
    # wo rows for this core's 4 heads, tiled [128, h, oc, 128]
    woT = nc.dram_tensor("woT", [128, HPC * NDC * 128], f16, kind="ExternalInput")
    lcos = nc.dram_tensor("lcos", [128, L], f16, kind="ExternalInput")
    lsin = nc.dram_tensor("lsin", [128, L], f16, kind="ExternalInput")
    qn = nc.dram_tensor("qn", [HD, 1], f32, kind="ExternalInput")
    kn = nc.dram_tensor("kn", [HD, 1], f32, kind="ExternalInput")
    # partial output: yT[oc, tok] = sum over this core's heads
    yT = nc.dram_tensor("yT", [D, L], f16, kind="ExternalOutput")

    with tile.TileContext(nc) as tc, ExitStack() as ctx, nc.allow_low_precision(
        reason="f16 storage; all matmul accumulation is fp32 PSUM"
    ):
        consts = ctx.enter_context(tc.tile_pool(name="consts", bufs=1))
        qkvp = ctx.enter_context(tc.tile_pool(name="qkvp", bufs=1))
        ropep = ctx.enter_context(tc.tile_pool(name="ropep", bufs=2))
        statp = ctx.enter_context(tc.tile_pool(name="statp", bufs=2))
        sap = ctx.enter_context(tc.tile_pool(name="sap", bufs=4))
        expp = ctx.enter_context(tc.tile_pool(name="expp", bufs=4))
        attp = ctx.enter_context(tc.tile_pool(name="attp", bufs=2))
        bcp = ctx.enter_context(tc.tile_pool(name="bcp", bufs=2))
        yp = ctx.enter_context(tc.tile_pool(name="yp", bufs=4))

        pbig = ctx.enter_context(tc.tile_pool(name="pbig", bufs=2, space="PSUM"))
        patt = ctx.enter_context(tc.tile_pool(name="patt", bufs=2, space="PSUM"))
        pscore = ctx.enter_context(tc.tile_pool(name="pscore", bufs=4, space="PSUM"))

        # ---- resident tensors. DMA priority order, split small so the
        # 16 queues parallelize and the first wave (kv cols x tt0/tt1)
        # streams right behind the DMA:
        #   kv w-cols + x tt0/tt1 per dc -> x tt2/tt3 -> q w-cols ----
        # Each DMA queue delivers ~15 GB/s (descriptor processing scales with
        # bytes); 16 queues ~ 240 GB/s aggregate. The first wave consumes one
        # dc (8 MMs) per 1.73us, i.e. ~380KB/dc of supply budget — so phase 1
        # carries ONLY w-kv cols + x tt0/tt1 per dc (384KB), partition-halved
        # so per-dc latency is ~8.6us with a ~1.6us/dc pipelined cadence.
        x_sb = consts.tile([128, NDC, L], f16)
        _xr = xT.ap().rearrange("(dc p) t -> p dc t", p=128)
        w_sb = consts.tile([128, NDC, QKV_COLS], f16)
        _wr = wqkv.ap().rearrange("(dc p) f -> p dc f", p=128)

        # dma_start issue costs ~600ns on the issuing sequencer; rotate the
        # issuing engine so 4 queues' worth issue in parallel at startup.
        _dma_engines = [nc.sync, nc.gpsimd, nc.scalar]
        _di = [0]

        def dma(out, in_):
            e = _dma_engines[_di[0] % 3]
            _di[0] += 1
            e.dma_start(out=out, in_=in_)

        for dc in range(NDC):
            nsp = 8 if dc < 2 else 2
            for s in range(nsp):
                p0, p1 = s * 128 // nsp, (s + 1) * 128 // nsp
                dma(w_sb[p0:p1, dc:dc + 1, 512:1024],
                    _wr[p0:p1, dc:dc + 1, 512:1024])
            for s in range(nsp):
                p0, p1 = s * 128 // nsp, (s + 1) * 128 // nsp
                dma(x_sb[p0:p1, dc:dc + 1, 0:2 * TT],
                    _xr[p0:p1, dc:dc + 1, 0:2 * TT])
        # phase 2: x tt2/tt3 (wave2 feed), then q weight cols (proj_tt(0) feed)
        for dc in range(NDC):
            for s in range(2):
                p0, p1 = s * 64, (s + 1) * 64
                dma(x_sb[p0:p1, dc:dc + 1, 2 * TT:L],
                    _xr[p0:p1, dc:dc + 1, 2 * TT:L])
        for dc in range(NDC):
            for s in range(2):
                p0, p1 = s * 64, (s + 1) * 64
                dma(w_sb[p0:p1, dc:dc + 1, 0:512],
                    _wr[p0:p1, dc:dc + 1, 0:512])
        # ---- constants ----
        ones_f = consts.tile([128, 1], f32)
        nc.vector.memset(ones_f, 1.0)
        ones = consts.tile([128, 1], f16)
        nc.vector.tensor_copy(out=ones, in_=ones_f)
        ident = consts.tile([128, 128], f16)
        make_identity(nc, ident)
        eps_t = consts.tile([1, 1], f32)
        nc.vector.memset(eps_t, EPS)
        neg8 = consts.tile([128, 1], f32)
        nc.vector.memset(neg8, -8.0)
        cos_sb = consts.tile([128, L], f16)
        nc.gpsimd.dma_start(out=cos_sb, in_=lcos[:, :])
        sin_sb = consts.tile([128, L], f16)
        nc.gpsimd.dma_start(out=sin_sb, in_=lsin[:, :])
        qn_sb = consts.tile([HD, 1], f32)
        nc.gpsimd.dma_start(out=qn_sb, in_=qn[:, :])
        kn_sb = consts.tile([HD, 1], f32)
        nc.gpsimd.dma_start(out=kn_sb, in_=kn[:, :])
        # wo tiles [128, h, oc, 128] (needed only at out-projection)
        wo_sb = consts.tile([128, HPC, NDC, 128], f16)
        _wor = woT.ap().rearrange("p (h oc m) -> p h oc m", h=HPC, oc=NDC)
        nc.sync.dma_start(out=wo_sb[:, 0:2, :, :], in_=_wor[:, 0:2, :, :])
        nc.sync.dma_start(out=wo_sb[:, 2:4, :, :], in_=_wor[:, 2:4, :, :])

        # activations
        qh_t = [qkvp.tile([128, L], f16, name=f"qh{h}") for h in range(HPC)]
        kh_t = [qkvp.tile([128, L], f16, name=f"kh{g}") for g in range(KPC)]
        v_t = [qkvp.tile([128, L // 128, HD], f16, name=f"v{g}") for g in range(KPC)]
        a_t = [qkvp.tile([128, L], f16, name=f"a{h}") for h in range(HPC)]

        def proj_post(tt, fc, rsrc, act_swap=False, pss=None):
            """RoPE + RMSNorm (q/k) or transpose (v) for a staged projection tile.

            RoPE with full-width ops: cos_sb = [cos;cos], sin_sb = [-sin;+sin];
            roped = rsrc*cos2 + swap(rsrc)*sin2."""
            pos0 = tt * TT
            if fc < 6:
                # RoPE preserves per-token sum of squares, so the rstd chain
                # runs off the pre-rope tile, in parallel with the rotation.
                sq = ropep.tile([128, TT], f16, tag="tb", name="sq")
                nc.vector.tensor_tensor(out=sq, in0=rsrc, in1=rsrc, op=mult)
                if pss is None:
                    pt = pscore.tile([1, TT], f32, tag="ps", name="pt")
                else:
                    pt = pscore.tile([1, TT], f32, tag="ps", name="ptsm")
                nc.tensor.matmul(pt, ones, sq, start=True, stop=True)
                lsrc = pt
                lnt = statp.tile([1, TT], f32, tag="statf")
                nc.scalar.activation(
                    out=lnt, in_=lsrc, func=Ln, bias=eps_t, scale=1.0 / HD
                )
                srd = statp.tile([1, TT], f16, tag="stath")
                nc.scalar.activation(out=srd, in_=lnt, func=Exp, scale=-0.5)
                pb = bcp.tile([128, TT], f16, tag="bc")
                nc.gpsimd.partition_broadcast(out_ap=pb, in_ap=srd)
                # rotation: ta = rsrc*[cos;cos]; tbs = rsrc*[+sin;-sin];
                # tb2 = swap(tbs) halves; roped = ta + tb2 (in place).
                ta = ropep.tile([128, TT], f16, tag="ta")
                nc.vector.tensor_tensor(
                    out=ta, in0=rsrc, in1=cos_sb[:, pos0:pos0 + TT], op=mult
                )
                tbs = ropep.tile([128, TT], f16, tag="tb", name="tbs")
                nc.vector.tensor_tensor(
                    out=tbs, in0=rsrc, in1=sin_sb[:, pos0:pos0 + TT], op=mult
                )
                tb2 = ropep.tile([128, TT], f16, tag="tb2")
                if act_swap:
                    # offload the half swaps to ACT in phases where it idles
                    nc.scalar.activation(
                        out=tb2[0:64, :], in_=tbs[64:128, :], func=Copy
                    )
                    nc.scalar.activation(
                        out=tb2[64:128, :], in_=tbs[0:64, :], func=Copy
                    )
                else:
                    nc.vector.tensor_copy(out=tb2[0:64, :], in_=tbs[64:128, :])
                    nc.vector.tensor_copy(out=tb2[64:128, :], in_=tbs[0:64, :])
                roped = ta
                nc.vector.tensor_tensor(out=roped, in0=ta, in1=tb2, op=add)
                w_head = qn_sb if fc < 4 else kn_sb
                if fc < 4:
                    dst = qh_t[fc][:, pos0:pos0 + TT]
                else:
                    dst = kh_t[fc - 4][:, pos0:pos0 + TT]
                nc.vector.scalar_tensor_tensor(
                    out=dst, in0=roped, scalar=w_head, in1=pb,
                    op0=mult, op1=mult,
                )
            else:
                g = fc - 6
                for i in range(TT // 128):
                    pt = pscore.tile([128, 128], f16, tag="ps")
                    nc.tensor.transpose(pt, rsrc[:, i * 128:(i + 1) * 128], ident)
                    nc.vector.tensor_copy(out=v_t[g][:, tt * 4 + i, :], in_=pt)

        def proj_chain(tt, fc):
            """Emit the 16-chunk projection chain for (tt, fc); return staged copy."""
            pos0 = tt * TT
            pp = pbig.tile([128, TT], f32, tag="pp", name=f"pp{fc}")
            for dc in range(NDC):
                nc.tensor.matmul(
                    pp,
                    w_sb[:, dc, fc * 128:(fc + 1) * 128],
                    x_sb[:, dc, pos0:pos0 + TT],
                    start=(dc == 0),
                    stop=(dc == NDC - 1),
                )
            rsrc = ropep.tile([128, TT], f16, tag="rsrc", bufs=4)
            nc.vector.tensor_copy(out=rsrc, in_=pp)
            return rsrc

        def proj_units(tt, fc):
            """Split one (tt, fc) projection chain into weave units:
            [8 MMs] [8 MMs + drain] [post]."""
            pos0 = tt * TT
            st = {}

            def ua():
                st["pp"] = pbig.tile([128, TT], f32, tag="pp", name=f"pw{fc}{tt}")
                for dc in range(8):
                    nc.tensor.matmul(
                        st["pp"],
                        w_sb[:, dc, fc * 128:(fc + 1) * 128],
                        x_sb[:, dc, pos0:pos0 + TT],
                        start=(dc == 0),
                        stop=False,
                    )

            def ub():
                for dc in range(8, NDC):
                    nc.tensor.matmul(
                        st["pp"],
                        w_sb[:, dc, fc * 128:(fc + 1) * 128],
                        x_sb[:, dc, pos0:pos0 + TT],
                        start=False,
                        stop=(dc == NDC - 1),
                    )
                rsrc = ropep.tile([128, TT], f16, tag="rsrc", bufs=4)
                nc.vector.tensor_copy(out=rsrc, in_=st["pp"])
                st["rsrc"] = rsrc

            def uc():
                # ssq via psmall: don't steal a score bank mid-attention
                proj_post(tt, fc, st["rsrc"])

            return [ua, ub, uc]

        def proj_wave(pairs, stage_dsts):
            """8 projection chains at once (one PSUM bank each, borrowed across
            pools) so the PE keeps pace with the initial per-dc x DMA stream.

            Results are staged into stage_dsts (idle a_t slices — unused until
            the attention finalizers, long after the posts consume them), so
            no staging-ring pressure. Returns deferred post closures."""
            slots = [
                pbig.tile([128, TT], f32, tag="pp", name="wv0"),
                pbig.tile([128, TT], f32, tag="pp", name="wv1"),
                patt.tile([128, TT], f32, tag="po", name="wv2"),
                patt.tile([128, TT], f32, tag="po", name="wv3"),
                pscore.tile([128, TT], f32, tag="ps", name="wv4"),
                pscore.tile([128, TT], f32, tag="ps", name="wv5"),
                pscore.tile([128, TT], f32, tag="ps", name="wv6"),
                pscore.tile([128, TT], f32, tag="ps", name="wv7"),
            ]
            assert len(pairs) == 8
            for dc in range(NDC):
                for i, (fc, tt) in enumerate(pairs):
                    pos0 = tt * TT
                    nc.tensor.matmul(
                        slots[i],
                        w_sb[:, dc, fc * 128:(fc + 1) * 128],
                        x_sb[:, dc, pos0:pos0 + TT],
                        start=(dc == 0),
                        stop=(dc == NDC - 1),
                    )
            staged = []
            for i, (fc, tt) in enumerate(pairs):
                dst = stage_dsts[i]
                # drain via ACT: it idles during the waves, DVE does not
                nc.scalar.activation(out=dst, in_=slots[i], func=Copy)
                staged.append((tt, fc, dst))
            return staged

        def proj_tt(tt, fcs, sprinkle=()):
            """Projection for one token tile: chains run two fc ahead of
            postprocessing; deferred wave posts sprinkle between chains."""
            sprinkle = list(sprinkle)
            pend = []
            for fc in fcs:
                rsrc = proj_chain(tt, fc)
                if sprinkle:
                    sprinkle.pop(0)()
                if sprinkle:
                    sprinkle.pop(0)()
                pend.append((fc, rsrc))
                if len(pend) > 1:
                    wfc, wrs = pend.pop(0)
                    proj_post(tt, wfc, wrs)
            for u in sprinkle:
                u()
            for wfc, wrs in pend:
                proj_post(tt, wfc, wrs)

        def outproj_chain(tt, oc, drain_act=False):
            """One 128-col partial out-projection chain + drain."""
            pos0 = tt * TT
            py = pbig.tile([128, TT], f32, tag="pp", name=f"py{oc % 2}")
            for h in range(HPC):
                nc.tensor.matmul(
                    py, wo_sb[:, h, oc, :], a_t[h][:, pos0:pos0 + TT],
                    start=(h == 0), stop=(h == HPC - 1),
                )
            yt = yp.tile([128, TT], f16, tag="y")
            if drain_act:
                nc.scalar.activation(out=yt, in_=py, func=Copy, scale=1.0)
            else:
                nc.vector.tensor_copy(out=yt, in_=py)
            nc.sync.dma_start(
                out=yT[oc * 128:(oc + 1) * 128, pos0:pos0 + TT], in_=yt
            )

        def oc_unit(tt, oc, drain_act=False):
            return lambda: outproj_chain(tt, oc, drain_act=drain_act)

        def att_tiles(jobs, weave=(), fin_prev=None):
            """Interleaved attention for (h, tqt) query tiles (same kv group).

            weave: list of closures (out-projection chains / q-proj units)
            emitted between key chunks to keep the PE fed while the ACT exp
            stream drains. fin_prev: finish closure of the previous pair,
            emitted after the first key chunks so its DVE-latency-bound
            normalize chain hides under this pair's score stream. Returns
            this pair's finish closure.

            exp tiles are paired [128, 2, TT] so the softmax running-sum adds
            run at [128, 2*TT] width, halving the DVE instruction count."""
            NK = L // 128
            weave = list(weave)
            state = []
            for h, tqt in jobs:
                g = h // 2
                qs = qh_t[h][:, tqt * TT:(tqt + 1) * TT]
                po = patt.tile([128, TT], f32, tag="po", name=f"po{h}{tqt}")
                sacc = sap.tile([128, 2, TT], f16, tag="sacc", bufs=3,
                                name=f"sa{h}{tqt}")
                state.append((h, g, tqt, qs, po, sacc, {}))
            nweave = len(weave)
            for tk in range(NK):
                for h, g, tqt, qs, po, sacc, st in state:
                    ps = pscore.tile([128, TT], f32, tag="ps")
                    nc.tensor.matmul(
                        ps, kh_t[g][:, tk * 128:(tk + 1) * 128], qs,
                        start=True, stop=True,
                    )
                    if tk % 2 == 0:
                        st["et2"] = expp.tile(
                            [128, 2, TT], f16, tag="expt", name=f"et{h}{tk}"
                        )
                    et = st["et2"][:, tk % 2, :]
                    # -8 exponent shift (softmax-invariant) keeps the po/v
                    # accumulator magnitudes small enough for f16 staging
                    nc.scalar.activation(
                        out=et, in_=ps, func=Exp, bias=neg8, scale=SCALE
                    )
                if tk == 0 and fin_prev is not None:
                    fin_prev()
                    fin_prev = None
                for h, g, tqt, qs, po, sacc, st in state:
                    et = st["et2"][:, tk % 2, :]
                    nc.tensor.matmul(
                        po, v_t[g][:, tk, :], et,
                        start=(tk == 0), stop=(tk == NK - 1),
                    )
                    if tk % 2 == 1:
                        pair = st["et2"][:, :, :]
                        sview = sacc[:, :, :]
                        if tk == 1:
                            nc.vector.tensor_copy(out=sview, in_=pair)
                        else:
                            nc.vector.tensor_tensor(
                                out=sview, in0=sview, in1=pair, op=add
                            )
                # weave filler chains uniformly across the key loop
                while weave and len(weave) > nweave * (NK - 1 - tk) // NK:
                    weave.pop(0)()

            def fin(tail_weave=()):
                pds = []
                for h, g, tqt, qs, po, sacc, st in state:
                    pd = pscore.tile([1, TT], f32, tag="ps", name="pd")
                    nc.tensor.matmul(
                        pd, ones, sacc[:, 0, :], start=True, stop=False
                    )
                    nc.tensor.matmul(
                        pd, ones, sacc[:, 1, :], start=False, stop=True
                    )
                    rdf = statp.tile([1, TT], f32, tag="statf")
                    nc.vector.reciprocal_approx_fast(out=rdf, in_=pd)
                    pds.append(rdf)
                for u in tail_weave:
                    u()
                # Batches of 2: po-releasing copies and broadcast issues first,
                # broadcast-dependent multiplies after, so the in-order DVE
                # queue doesn't stall on a GpSimd broadcast before freeing po.
                pairs2 = [(s[0], s[2], s[4], rdf) for s, rdf in zip(state, pds)]
                for j0 in range(0, len(pairs2), 2):
                    staged2 = []
                    for h, tqt, po, rdf in pairs2[j0:j0 + 2]:
                        rd = statp.tile([1, TT], f16, tag="stath")
                        nc.vector.tensor_copy(out=rd, in_=rdf)
                        pb = bcp.tile([128, TT], f16, tag="bc")
                        nc.gpsimd.partition_broadcast(out_ap=pb, in_ap=rd)
                        o_sb = attp.tile([128, TT], f16, tag="att")
                        nc.vector.tensor_copy(out=o_sb, in_=po)
                        staged2.append((h, tqt, pb, o_sb))
                    for h, tqt, pb, o_sb in staged2:
                        nc.vector.tensor_tensor(
                            out=a_t[h][:, tqt * TT:(tqt + 1) * TT],
                            in0=o_sb, in1=pb, op=mult,
                        )

            return fin

        def outproj_tt(tt, drain_act=False):
            for oc in range(NDC):
                outproj_chain(tt, oc, drain_act=drain_act and oc % 2 == 0)

        # ---- schedule ----
        # KV proj waves first (posts deferred into the next MM stream);
        # q proj tt0 dense; q proj tt1-3 woven into the early attention pairs
        # as half-chain units; the out-projection woven into later pairs so
        # the PE never starves while the ACT exp stream drains.
        def wave_dsts(a_lo, a_hi):
            return [a_t[a_lo][:, s * TT:(s + 1) * TT] for s in range(4)] + [
                a_t[a_hi][:, s * TT:(s + 1) * TT] for s in range(4)
            ]

        st1 = proj_wave(
            [(4, 0), (5, 0), (6, 0), (7, 0), (4, 1), (5, 1), (6, 1), (7, 1)],
            stage_dsts=wave_dsts(0, 1),
        )
        st2 = proj_wave(
            [(4, 2), (5, 2), (6, 2), (7, 2), (4, 3), (5, 3), (6, 3), (7, 3)],
            stage_dsts=wave_dsts(2, 3),
        )

        def wpost(item, **kw):
            tt, fc, r = item
            proj_post(tt, fc, r, **kw)

        # wave1 posts, v-transposes first (their data staged long before so
        # the PE never waits on fresh DVE); k-posts share a packed ssq tile
        for i in (2, 3, 6, 7):
            wpost(st1[i])
        for i in (0, 1, 4, 5):
            wpost(st1[i], act_swap=True)
        # proj_tt(0) by hand: wave2's v-posts + 2 k-posts sprinkle between
        # chains; the (4,3)/(5,3) k-posts and the q2/q3 posts defer into
        # pair A's weave (ACT slack there; their consumers come later).
        r0 = proj_chain(0, 0)
        wpost(st2[2])                     # v (6,2)
        wpost(st2[3])                     # v (7,2)
        r1 = proj_chain(0, 1)
        wpost(st2[6])                     # v (6,3)
        wpost(st2[7])                     # v (7,3)
        proj_post(0, 0, r0, act_swap=True)
        r2 = proj_chain(0, 2)
        wpost(st2[0], act_swap=True)   # k (4,2)
        wpost(st2[1], act_swap=True)   # k (5,2)
        proj_post(0, 1, r1, act_swap=True)
        r3 = proj_chain(0, 3)

        w_a = [lambda: wpost(st2[4]),            # k (4,3)
               lambda: wpost(st2[5]),            # k (5,3)
               lambda: proj_post(0, 2, r2),
               lambda: proj_post(0, 3, r3)]
        w_a += proj_units(1, 0) + proj_units(1, 1)
        fin = att_tiles([(0, 0), (1, 0)], weave=w_a)
        w_b = proj_units(1, 2) + proj_units(1, 3)
        fin = att_tiles([(2, 0), (3, 0)], weave=w_b, fin_prev=fin)
        w_c = proj_units(2, 0) + proj_units(2, 1)
        fin = att_tiles([(0, 1), (1, 1)], weave=w_c, fin_prev=fin)
        w_d = proj_units(2, 2) + proj_units(2, 3)
        fin = att_tiles([(2, 1), (3, 1)], weave=w_d, fin_prev=fin)
        w_e = proj_units(3, 0) + proj_units(3, 1) + [oc_unit(0, 0), oc_unit(0, 1)]
        fin = att_tiles([(0, 2), (1, 2)], weave=w_e, fin_prev=fin)
        w_f = proj_units(3, 2) + proj_units(3, 3) + [oc_unit(0, 2), oc_unit(0, 3)]
        fin = att_tiles([(2, 2), (3, 2)], weave=w_f, fin_prev=fin)
        w_g = [oc_unit(0, oc) for oc in range(4, 10)] + [oc_unit(1, oc) for oc in range(0, 2)]
        fin = att_tiles([(0, 3), (1, 3)], weave=w_g, fin_prev=fin)
        w_h = [oc_unit(0, oc) for oc in range(10, 16)] + [oc_unit(1, oc) for oc in range(2, 4)]
        fin = att_tiles([(2, 3), (3, 3)], weave=w_h, fin_prev=fin)
        fin(tail_weave=[oc_unit(1, oc) for oc in range(4, 8)])
        # dense tail: alternate drain engines so neither DVE nor ACT
        # serializes the chain pipeline
        tail = [(1, oc) for oc in range(8, 16)]
        tail += [(2, oc) for oc in range(NDC)]
        tail += [(3, oc) for oc in range(NDC)]
        for i, (tt, oc) in enumerate(tail):
            outproj_chain(tt, oc, drain_act=i % 2 == 0)

    nc.finalize()
    return nc


def kernel(x, wq, wk, wv, wo, qn_w, kn_w):
    from concourse.bass_utils import run_bass_kernel_spmd

    if "nc" not in _CACHE:
        _CACHE["nc"] = _build_nc()
    nc = _CACHE["nc"]

    x = np.asarray(x, dtype=np.float32)
    wq = np.asarray(wq, dtype=np.float32)
    wk = np.asarray(wk, dtype=np.float32)
    wv = np.asarray(wv, dtype=np.float32)
    wo = np.asarray(wo, dtype=np.float32)
    qn_w = np.asarray(qn_w, dtype=np.float32).reshape(HD, 1).copy()
    kn_w = np.asarray(kn_w, dtype=np.float32).reshape(HD, 1).copy()

    cos2, sin2 = _rope_tables()
    cos2 = cos2.astype(np.float16)
    sin2 = sin2.astype(np.float16)

    in_maps = []
    for c in range(NCORES):
        bc, hc4 = divmod(c, 4)
        xT_c = np.ascontiguousarray(x[bc].T.astype(np.float16))
        wqkv_c = np.ascontiguousarray(
            np.concatenate(
                [
                    wq[:, hc4 * HPC * HD:(hc4 + 1) * HPC * HD],
                    wk[:, hc4 * KPC * HD:(hc4 + 1) * KPC * HD],
                    wv[:, hc4 * KPC * HD:(hc4 + 1) * KPC * HD],
                ],
                axis=1,
            ).astype(np.float16)
        )
        # wo rows for this core's heads -> [128, h*oc*128]
        wo_sl = wo[hc4 * HPC * HD:(hc4 + 1) * HPC * HD, :].astype(np.float16)
        wo_t = np.ascontiguousarray(
            wo_sl.reshape(HPC, 128, NDC, 128).transpose(1, 0, 2, 3).reshape(
                128, HPC * NDC * 128
            )
        )
        in_maps.append(
            {
                "xT": xT_c,
                "wqkv": wqkv_c,
                "woT": wo_t,
                "lcos": cos2,
                "lsin": sin2,
                "qn": qn_w,
                "kn": kn_w,
            }
        )

    trace = bool(_CACHE.get("trace"))
    r = run_bass_kernel_spmd(
        nc, in_maps, core_ids=list(range(NCORES)), trace=trace
    )
    _CACHE["last_result"] = r

    y = np.empty((B, L, D), dtype=np.float32)
    for bc in range(B):
        acc = r.results[bc * 4]["yT"].astype(np.float32)
        for hc4 in range(1, 4):
            acc += r.results[bc * 4 + hc4]["yT"]
        y[bc] = acc.T
    return y


# revision 50
# speedup vs baseline: 1.5527x; 1.0581x over previous
"""Trainium2 Bass kernel for GQA attention (B=2, L=2048, D=2048, H=16, KV=8, HD=128).

Sharding: 2-way data-parallel over batch x 4-way tensor-parallel over heads
(KV-head groups intact). Each core handles one batch and 4 query heads
(2 KV heads): QKV projection + RoPE + RMSNorm + flash-style attention +
a PARTIAL output projection over its 4 heads' rows of wo (row-sharded wo).
The host gather SUMS the 4 partial [D, L] outputs per batch — no on-device
collective. Avoiding collective_compute keeps the PE at full clock.

All heavy matmuls are fp16 with fp32 PSUM accumulation. rstd and softmax
denominators use the Ln/Exp one-table trick and PE ones/broadcast matmuls.

v2 schedule: full-width RoPE ops (host-precomputed [cos;cos] and [-sin;+sin]
tables), paired exp tiles for half-count softmax accumulation adds, and
q-projection half-chains woven into the early attention pairs so the PE
stays fed while the ACT exp stream is the per-pair wall.
"""
import numpy as np

B, L, D = 2, 2048, 2048
H, KV, HD = 16, 8, 128
NCORES = 8
HPC = 4              # query heads per core
KPC = 2              # kv heads per core
EPS = 1e-5
ROPE_BASE = 10000.0
SCALE = HD ** -0.5

TT = 512             # token tile (free dim)
NTT = L // TT        # 4 token tiles
NDC = D // 128       # 16 contraction chunks in qkv proj
NFC = 8              # 4 q + 2 k + 2 v column chunks of 128
QKV_COLS = NFC * 128

_CACHE = {}


def _rope_tables():
    """[128, L] cos2=[cos;cos] and sin2=[-sin;+sin] LUTs, matching the jax
    reference numerics (f32, cpu)."""
    import jax
    import jax.numpy as jnp

    cpu = jax.devices("cpu")[0]
    with jax.default_device(cpu):
        base = ROPE_BASE * 1.0 ** (HD / (HD - 2))
        freqs = base ** (jnp.arange(0, HD, 2, dtype=jnp.float32) / HD)   # [64]
        pos = jnp.arange(L, dtype=jnp.float32)                           # [L]
        angles = pos[:, None] * freqs[None, :]                           # [L, 64]
        cos = np.asarray(jnp.cos(angles), dtype=np.float32).T            # [64, L]
        sin = np.asarray(jnp.sin(angles), dtype=np.float32).T
    cos2 = np.concatenate([cos, cos], axis=0)     # [128, L]
    sin2 = np.concatenate([sin, -sin], axis=0)    # [128, L]
    return np.ascontiguousarray(cos2), np.ascontiguousarray(sin2)


def _build_nc():
    import concourse.bass as bass
    import concourse.tile as tile
    import concourse.mybir as mybir
    from concourse import bacc
    from concourse.masks import make_identity
    from contextlib import ExitStack

    f32 = mybir.dt.float32
    f16 = mybir.dt.float16
    Exp = mybir.ActivationFunctionType.Exp
    Ln = mybir.ActivationFunctionType.Ln
    Copy = mybir.ActivationFunctionType.Copy
    mult = mybir.AluOpType.mult
    add = mybir.AluOpType.add

    from concourse import bacc as _bacc_mod

    if not getattr(_bacc_mod, "_act_table_patch", False):
        _orig_get = _bacc_mod.get_activation_tables

        def _patched_get(arch):
            t = _orig_get(arch)
            exp = mybir.ActivationFunctionType.Exp
            ln = mybir.ActivationFunctionType.Ln
            for name, funcs in t.items():
                if name != "natural_log_exp_and_others":
                    funcs.discard(exp)
                    funcs.discard(ln)
            return t

        _bacc_mod.get_activation_tables = _patched_get
        _bacc_mod._act_table_patch = True

    nc = bacc.Bacc(num_devices=NCORES)

    # per-core inputs (host pre-sliced)
    xT = nc.dram_tensor("xT", [D, L], f16, kind="ExternalInput")
    wqkv = nc.dram_tensor("wqkv", [D, QKV_COLS], f16, kind="ExternalInput")
    # wo rows for this core's 4 heads, tiled [128, h, oc, 128]
    woT = nc.dram_tensor("woT", [128, HPC * NDC * 128], f16, kind="ExternalInput")
    lcos = nc.dram_tensor("lcos", [128, L], f16, kind="ExternalInput")
    lsin = nc.dram_tensor("lsin", [128, L], f16, kind="ExternalInput")
    qn = nc.dram_tensor("qn", [HD, 1], f32, kind="ExternalInput")
    kn = nc.dram_tensor("kn", [HD, 1], f32, kind="ExternalInput")
    # partial output: yT[oc, tok] = sum over this core's heads
    yT = nc.dram_tensor("yT", [D, L], f16, kind="ExternalOutput")

    with tile.TileContext(nc) as tc, ExitStack() as ctx, nc.allow_low_precision(
        reason="f16 storage; all matmul accumulation is fp32 PSUM"
    ):
        consts = ctx.enter_context(tc.tile_pool(name="consts", bufs=1))
        qkvp = ctx.enter_context(tc.tile_pool(name="qkvp", bufs=1))
        ropep = ctx.enter_context(tc.tile_pool(name="ropep", bufs=2))
        statp = ctx.enter_context(tc.tile_pool(name="statp", bufs=2))
        sap = ctx.enter_context(tc.tile_pool(name="sap", bufs=4))
        expp = ctx.enter_context(tc.tile_pool(name="expp", bufs=4))
        attp = ctx.enter_context(tc.tile_pool(name="attp", bufs=2))
        bcp = ctx.enter_context(tc.tile_pool(name="bcp", bufs=2))
        yp = ctx.enter_context(tc.tile_pool(name="yp", bufs=4))

        pbig = ctx.enter_context(tc.tile_pool(name="pbig", bufs=2, space="PSUM"))
        patt = ctx.enter_context(tc.tile_pool(name="patt", bufs=2, space="PSUM"))
        pscore = ctx.enter_context(tc.tile_pool(name="pscore", bufs=4, space="PSUM"))

        # ---- resident tensors. DMA priority order, split small so the
        # 16 queues parallelize and the first wave (kv cols x tt0/tt1)
        # streams right behind the DMA:
        #   kv w-cols + x tt0/tt1 per dc -> x tt2/tt3 -> q w-cols ----
        # Each DMA queue delivers ~15 GB/s (descriptor processing scales with
        # bytes); 16 queues ~ 240 GB/s aggregate. The first wave consumes one
        # dc (8 MMs) per 1.73us, i.e. ~380KB/dc of supply budget — so phase 1
        # carries ONLY w-kv cols + x tt0/tt1 per dc (384KB), partition-halved
        # so per-dc latency is ~8.6us with a ~1.6us/dc pipelined cadence.
        x_sb = consts.tile([128, NDC, L], f16)
        _xr = xT.ap().rearrange("(dc p) t -> p dc t", p=128)
        w_sb = consts.tile([128, NDC, QKV_COLS], f16)
        _wr = wqkv.ap().rearrange("(dc p) f -> p dc f", p=128)

        # dma_start issue costs ~600ns on the issuing sequencer; rotate the
        # issuing engine so 4 queues' worth issue in parallel at startup.
        _dma_engines = [nc.sync, nc.gpsimd, nc.scalar]
        _di = [0]

        def dma(out, in_):
            e = _dma_engines[_di[0] % 3]
            _di[0] += 1
            e.dma_start(out=out, in_=in_)

        for dc in range(NDC):
            nsp = 8 if dc < 2 else 2
            for s in range(nsp):
                p0, p1 = s * 128 // nsp, (s + 1) * 128 // nsp
                dma(w_sb[p0:p1, dc:dc + 1, 512:1024],
                    _wr[p0:p1, dc:dc + 1, 512:1024])
            for s in range(nsp):
                p0, p1 = s * 128 // nsp, (s + 1) * 128 // nsp
                dma(x_sb[p0:p1, dc:dc + 1, 0:2 * TT],
                    _xr[p0:p1, dc:dc + 1, 0:2 * TT])
        # phase 2: x tt2/tt3 (wave2 feed), then q weight cols (proj_tt(0) feed)
        for dc in range(NDC):
            for s in range(2):
                p0, p1 = s * 64, (s + 1) * 64
                dma(x_sb[p0:p1, dc:dc + 1, 2 * TT:L],
                    _xr[p0:p1, dc:dc + 1, 2 * TT:L])
        for dc in range(NDC):
            for s in range(2):
                p0, p1 = s * 64, (s + 1) * 64
                dma(w_sb[p0:p1, dc:dc + 1, 0:512],
                    _wr[p0:p1, dc:dc + 1, 0:512])
        # ---- constants ----
        ones_f = consts.tile([128, 1], f32)
        nc.vector.memset(ones_f, 1.0)
        ones = consts.tile([128, 1], f16)
        nc.vector.tensor_copy(out=ones, in_=ones_f)
        ident = consts.tile([128, 128], f16)
        make_identity(nc, ident)
        eps_t = consts.tile([1, 1], f32)
        nc.vector.memset(eps_t, EPS)
        neg8 = consts.tile([128, 1], f32)
        nc.vector.memset(neg8, -8.0)
        cos_sb = consts.tile([128, L], f16)
        nc.gpsimd.dma_start(out=cos_sb, in_=lcos[:, :])
        sin_sb = consts.tile([128, L], f16)
        nc.gpsimd.dma_start(out=sin_sb, in_=lsin[:, :])
        qn_sb = consts.tile([HD, 1], f32)
        nc.gpsimd.dma_start(out=qn_sb, in_=qn[:, :])
        kn_sb = consts.tile([HD, 1], f32)
        nc.gpsimd.dma_start(out=kn_sb, in_=kn[:, :])
        # wo tiles [128, h, oc, 128] (needed only at out-projection)
        wo_sb = consts.tile([128, HPC, NDC, 128], f16)
        _wor = woT.ap().rearrange("p (h oc m) -> p h oc m", h=HPC, oc=NDC)
        nc.sync.dma_start(out=wo_sb[:, 0:2, :, :], in_=_wor[:, 0:2, :, :])
        nc.sync.dma_start(out=wo_sb[:, 2:4, :, :], in_=_wor[:, 2:4, :, :])

        # activations
        qh_t = [qkvp.tile([128, L], f16, name=f"qh{h}") for h in range(HPC)]
        kh_t = [qkvp.tile([128, L], f16, name=f"kh{g}") for g in range(KPC)]
        v_t = [qkvp.tile([128, L // 128, HD], f16, name=f"v{g}") for g in range(KPC)]
        a_t = [qkvp.tile([128, L], f16, name=f"a{h}") for h in range(HPC)]

        def proj_post(tt, fc, rsrc, act_swap=False, pss=None):
            """RoPE + RMSNorm (q/k) or transpose (v) for a staged projection tile.

            RoPE with full-width ops: cos_sb = [cos;cos], sin_sb = [-sin;+sin];
            roped = rsrc*cos2 + swap(rsrc)*sin2."""
            pos0 = tt * TT
            if fc < 6:
                # RoPE preserves per-token sum of squares, so the rstd chain
                # runs off the pre-rope tile, in parallel with the rotation.
                sq = ropep.tile([128, TT], f16, tag="tb", name="sq")
                nc.vector.tensor_tensor(out=sq, in0=rsrc, in1=rsrc, op=mult)
                if pss is None:
                    pt = pscore.tile([1, TT], f32, tag="ps", name="pt")
                else:
                    pt = pscore.tile([1, TT], f32, tag="ps", name="ptsm")
                nc.tensor.matmul(pt, ones, sq, start=True, stop=True)
                lsrc = pt
                lnt = statp.tile([1, TT], f32, tag="statf")
                nc.scalar.activation(
                    out=lnt, in_=lsrc, func=Ln, bias=eps_t, scale=1.0 / HD
                )
                srd = statp.tile([1, TT], f16, tag="stath")
                nc.scalar.activation(out=srd, in_=lnt, func=Exp, scale=-0.5)
                pb = bcp.tile([128, TT], f16, tag="bc")
                nc.gpsimd.partition_broadcast(out_ap=pb, in_ap=srd)
                # rotation: ta = rsrc*[cos;cos]; tbs = rsrc*[+sin;-sin];
                # tb2 = swap(tbs) halves; roped = ta + tb2 (in place).
                ta = ropep.tile([128, TT], f16, tag="ta")
                nc.vector.tensor_tensor(
                    out=ta, in0=rsrc, in1=cos_sb[:, pos0:pos0 + TT], op=mult
                )
                tbs = ropep.tile([128, TT], f16, tag="tb", name="tbs")
                nc.vector.tensor_tensor(
                    out=tbs, in0=rsrc, in1=sin_sb[:, pos0:pos0 + TT], op=mult
                )
                tb2 = ropep.tile([128, TT], f16, tag="tb2")
                if act_swap:
                    # offload the half swaps to ACT in phases where it idles
                    nc.scalar.activation(
                        out=tb2[0:64, :], in_=tbs[64:128, :], func=Copy
                    )
                    nc.scalar.activation(
                        out=tb2[64:128, :], in_=tbs[0:64, :], func=Copy
                    )
                else:
                    nc.vector.tensor_copy(out=tb2[0:64, :], in_=tbs[64:128, :])
                    nc.vector.tensor_copy(out=tb2[64:128, :], in_=tbs[0:64, :])
                roped = ta
                nc.vector.tensor_tensor(out=roped, in0=ta, in1=tb2, op=add)
                w_head = qn_sb if fc < 4 else kn_sb
                if fc < 4:
                    dst = qh_t[fc][:, pos0:pos0 + TT]
                else:
                    dst = kh_t[fc - 4][:, pos0:pos0 + TT]
                nc.vector.scalar_tensor_tensor(
                    out=dst, in0=roped, scalar=w_head, in1=pb,
                    op0=mult, op1=mult,
                )
            else:
                g = fc - 6
                for i in range(TT // 128):
                    pt = pscore.tile([128, 128], f16, tag="ps")
                    nc.tensor.transpose(pt, rsrc[:, i * 128:(i + 1) * 128], ident)
                    nc.vector.tensor_copy(out=v_t[g][:, tt * 4 + i, :], in_=pt)

        def proj_chain(tt, fc):
            """Emit the 16-chunk projection chain for (tt, fc); return staged copy."""
            pos0 = tt * TT
            pp = pbig.tile([128, TT], f32, tag="pp", name=f"pp{fc}")
            for dc in range(NDC):
                nc.tensor.matmul(
                    pp,
                    w_sb[:, dc, fc * 128:(fc + 1) * 128],
                    x_sb[:, dc, pos0:pos0 + TT],
                    start=(dc == 0),
                    stop=(dc == NDC - 1),
                )
            rsrc = ropep.tile([128, TT], f16, tag="rsrc", bufs=4)
            nc.vector.tensor_copy(out=rsrc, in_=pp)
            return rsrc

        def proj_units(tt, fc):
            """Split one (tt, fc) projection chain into weave units:
            [8 MMs] [8 MMs + drain] [post]."""
            pos0 = tt * TT
            st = {}

            def ua():
                st["pp"] = pbig.tile([128, TT], f32, tag="pp", name=f"pw{fc}{tt}")
                for dc in range(8):
                    nc.tensor.matmul(
                        st["pp"],
                        w_sb[:, dc, fc * 128:(fc + 1) * 128],
                        x_sb[:, dc, pos0:pos0 + TT],
                        start=(dc == 0),
                        stop=False,
                    )

            def ub():
                for dc in range(8, NDC):
                    nc.tensor.matmul(
                        st["pp"],
                        w_sb[:, dc, fc * 128:(fc + 1) * 128],
                        x_sb[:, dc, pos0:pos0 + TT],
                        start=False,
                        stop=(dc == NDC - 1),
                    )
                rsrc = ropep.tile([128, TT], f16, tag="rsrc", bufs=4)
                nc.vector.tensor_copy(out=rsrc, in_=st["pp"])
                st["rsrc"] = rsrc

            def uc():
                # ssq via psmall: don't steal a score bank mid-attention
                proj_post(tt, fc, st["rsrc"])

            return [ua, ub, uc]

        def proj_wave(pairs, stage_dsts):
            """8 projection chains at once (one PSUM bank each, borrowed across
            pools) so the PE keeps pace with the initial per-dc x DMA stream.

            Results are staged into stage_dsts (idle a_t slices — unused until
            the attention finalizers, long after the posts consume them), so
            no staging-ring pressure. Returns deferred post closures."""
            slots = [
                pbig.tile([128, TT], f32, tag="pp", name="wv0"),
                pbig.tile([128, TT], f32, tag="pp", name="wv1"),
                patt.tile([128, TT], f32, tag="po", name="wv2"),
                patt.tile([128, TT], f32, tag="po", name="wv3"),
                pscore.tile([128, TT], f32, tag="ps", name="wv4"),
                pscore.tile([128, TT], f32, tag="ps", name="wv5"),
                pscore.tile([128, TT], f32, tag="ps", name="wv6"),
                pscore.tile([128, TT], f32, tag="ps", name="wv7"),
            ]
            assert len(pairs) == 8
            for dc in range(NDC):
                for i, (fc, tt) in enumerate(pairs):
                    pos0 = tt * TT
                    nc.tensor.matmul(
                        slots[i],
                        w_sb[:, dc, fc * 128:(fc + 1) * 128],
                        x_sb[:, dc, pos0:pos0 + TT],
                        start=(dc == 0),
                        stop=(dc == NDC - 1),
                    )
            staged = []
            for i, (fc, tt) in enumerate(pairs):
                dst = stage_dsts[i]
                nc.vector.tensor_copy(out=dst, in_=slots[i])
                staged.append((tt, fc, dst))
            return staged

        def proj_tt(tt, fcs, sprinkle=()):
            """Projection for one token tile: chains run two fc ahead of
            postprocessing; deferred wave posts sprinkle between chains."""
            sprinkle = list(sprinkle)
            pend = []
            for fc in fcs:
                rsrc = proj_chain(tt, fc)
                if sprinkle:
                    sprinkle.pop(0)()
                if sprinkle:
                    sprinkle.pop(0)()
                pend.append((fc, rsrc))
                if len(pend) > 1:
                    wfc, wrs = pend.pop(0)
                    proj_post(tt, wfc, wrs)
            for u in sprinkle:
                u()
            for wfc, wrs in pend:
                proj_post(tt, wfc, wrs)

        def outproj_chain(tt, oc, drain_act=False):
            """One 128-col partial out-projection chain + drain."""
            pos0 = tt * TT
            py = pbig.tile([128, TT], f32, tag="pp", name=f"py{oc % 2}")
            for h in range(HPC):
                nc.tensor.matmul(
                    py, wo_sb[:, h, oc, :], a_t[h][:, pos0:pos0 + TT],
                    start=(h == 0), stop=(h == HPC - 1),
                )
            yt = yp.tile([128, TT], f16, tag="y")
            if drain_act:
                nc.scalar.activation(out=yt, in_=py, func=Copy, scale=1.0)
            else:
                nc.vector.tensor_copy(out=yt, in_=py)
            nc.sync.dma_start(
                out=yT[oc * 128:(oc + 1) * 128, pos0:pos0 + TT], in_=yt
            )

        def oc_unit(tt, oc, drain_act=False):
            return lambda: outproj_chain(tt, oc, drain_act=drain_act)

        def att_tiles(jobs, weave=(), fin_prev=None):
            """Interleaved attention for (h, tqt) query tiles (same kv group).

            weave: list of closures (out-projection chains / q-proj units)
            emitted between key chunks to keep the PE fed while the ACT exp
            stream drains. fin_prev: finish closure of the previous pair,
            emitted after the first key chunks so its DVE-latency-bound
            normalize chain hides under this pair's score stream. Returns
            this pair's finish closure.

            exp tiles are paired [128, 2, TT] so the softmax running-sum adds
            run at [128, 2*TT] width, halving the DVE instruction count."""
            NK = L // 128
            weave = list(weave)
            state = []
            for h, tqt in jobs:
                g = h // 2
                qs = qh_t[h][:, tqt * TT:(tqt + 1) * TT]
                po = patt.tile([128, TT], f32, tag="po", name=f"po{h}{tqt}")
                sacc = sap.tile([128, 2, TT], f16, tag="sacc", bufs=3,
                                name=f"sa{h}{tqt}")
                state.append((h, g, tqt, qs, po, sacc, {}))
            nweave = len(weave)
            for tk in range(NK):
                for h, g, tqt, qs, po, sacc, st in state:
                    ps = pscore.tile([128, TT], f32, tag="ps")
                    nc.tensor.matmul(
                        ps, kh_t[g][:, tk * 128:(tk + 1) * 128], qs,
                        start=True, stop=True,
                    )
                    if tk % 2 == 0:
                        st["et2"] = expp.tile(
                            [128, 2, TT], f16, tag="expt", name=f"et{h}{tk}"
                        )
                    et = st["et2"][:, tk % 2, :]
                    # -8 exponent shift (softmax-invariant) keeps the po/v
                    # accumulator magnitudes small enough for f16 staging
                    nc.scalar.activation(
                        out=et, in_=ps, func=Exp, bias=neg8, scale=SCALE
                    )
                if tk == 0 and fin_prev is not None:
                    fin_prev()
                    fin_prev = None
                for h, g, tqt, qs, po, sacc, st in state:
                    et = st["et2"][:, tk % 2, :]
                    nc.tensor.matmul(
                        po, v_t[g][:, tk, :], et,
                        start=(tk == 0), stop=(tk == NK - 1),
                    )
                    if tk % 2 == 1:
                        pair = st["et2"][:, :, :]
                        sview = sacc[:, :, :]
                        if tk == 1:
                            nc.vector.tensor_copy(out=sview, in_=pair)
                        else:
                            nc.vector.tensor_tensor(
                                out=sview, in0=sview, in1=pair, op=add
                            )
                # weave filler chains uniformly across the key loop
                while weave and len(weave) > nweave * (NK - 1 - tk) // NK:
                    weave.pop(0)()

            def fin(tail_weave=()):
                pds = []
                for h, g, tqt, qs, po, sacc, st in state:
                    pd = pscore.tile([1, TT], f32, tag="ps", name="pd")
                    nc.tensor.matmul(
                        pd, ones, sacc[:, 0, :], start=True, stop=False
                    )
                    nc.tensor.matmul(
                        pd, ones, sacc[:, 1, :], start=False, stop=True
                    )
                    rdf = statp.tile([1, TT], f32, tag="statf")
                    nc.vector.reciprocal_approx_fast(out=rdf, in_=pd)
                    pds.append(rdf)
                for u in tail_weave:
                    u()
                # Batches of 2: po-releasing copies and broadcast issues first,
                # broadcast-dependent multiplies after, so the in-order DVE
                # queue doesn't stall on a GpSimd broadcast before freeing po.
                pairs2 = [(s[0], s[2], s[4], rdf) for s, rdf in zip(state, pds)]
                for j0 in range(0, len(pairs2), 2):
                    staged2 = []
                    for h, tqt, po, rdf in pairs2[j0:j0 + 2]:
                        rd = statp.tile([1, TT], f16, tag="stath")
                        nc.vector.tensor_copy(out=rd, in_=rdf)
                        pb = bcp.tile([128, TT], f16, tag="bc")
                        nc.gpsimd.partition_broadcast(out_ap=pb, in_ap=rd)
                        o_sb = attp.tile([128, TT], f16, tag="att")
                        nc.vector.tensor_copy(out=o_sb, in_=po)
                        staged2.append((h, tqt, pb, o_sb))
                    for h, tqt, pb, o_sb in staged2:
                        nc.vector.tensor_tensor(
                            out=a_t[h][:, tqt * TT:(tqt + 1) * TT],
                            in0=o_sb, in1=pb, op=mult,
                        )

            return fin

        def outproj_tt(tt, drain_act=False):
            for oc in range(NDC):
                outproj_chain(tt, oc, drain_act=drain_act and oc % 2 == 0)

        # ---- schedule ----
        # KV proj waves first (posts deferred into the next MM stream);
        # q proj tt0 dense; q proj tt1-3 woven into the early attention pairs
        # as half-chain units; the out-projection woven into later pairs so
        # the PE never starves while the ACT exp stream drains.
        def wave_dsts(a_lo, a_hi):
            return [a_t[a_lo][:, s * TT:(s + 1) * TT] for s in range(4)] + [
                a_t[a_hi][:, s * TT:(s + 1) * TT] for s in range(4)
            ]

        st1 = proj_wave(
            [(4, 0), (5, 0), (6, 0), (7, 0), (4, 1), (5, 1), (6, 1), (7, 1)],
            stage_dsts=wave_dsts(0, 1),
        )
        st2 = proj_wave(
            [(4, 2), (5, 2), (6, 2), (7, 2), (4, 3), (5, 3), (6, 3), (7, 3)],
            stage_dsts=wave_dsts(2, 3),
        )

        def wpost(item, **kw):
            tt, fc, r = item
            proj_post(tt, fc, r, **kw)

        # wave1 posts, v-transposes first (their data staged long before so
        # the PE never waits on fresh DVE); k-posts share a packed ssq tile
        for i in (2, 3, 6, 7):
            wpost(st1[i])
        for i in (0, 1, 4, 5):
            wpost(st1[i], act_swap=True)
        # proj_tt(0) by hand: wave2's v-posts + 2 k-posts sprinkle between
        # chains; the (4,3)/(5,3) k-posts and the q2/q3 posts defer into
        # pair A's weave (ACT slack there; their consumers come later).
        r0 = proj_chain(0, 0)
        wpost(st2[2])                     # v (6,2)
        wpost(st2[3])                     # v (7,2)
        r1 = proj_chain(0, 1)
        wpost(st2[6])                     # v (6,3)
        wpost(st2[7])                     # v (7,3)
        proj_post(0, 0, r0, act_swap=True)
        r2 = proj_chain(0, 2)
        wpost(st2[0], act_swap=True)   # k (4,2)
        wpost(st2[1], act_swap=True)   # k (5,2)
        proj_post(0, 1, r1, act_swap=True)
        r3 = proj_chain(0, 3)

        w_a = [lambda: wpost(st2[4]),            # k (4,3)
               lambda: wpost(st2[5]),            # k (5,3)
               lambda: proj_post(0, 2, r2),
               lambda: proj_post(0, 3, r3)]
        w_a += proj_units(1, 0) + proj_units(1, 1)
        fin = att_tiles([(0, 0), (1, 0)], weave=w_a)
        w_b = proj_units(1, 2) + proj_units(1, 3)
        fin = att_tiles([(2, 0), (3, 0)], weave=w_b, fin_prev=fin)
        w_c = proj_units(2, 0) + proj_units(2, 1)
        fin = att_tiles([(0, 1), (1, 1)], weave=w_c, fin_prev=fin)
        w_d = proj_units(2, 2) + proj_units(2, 3)
        fin = att_tiles([(2, 1), (3, 1)], weave=w_d, fin_prev=fin)
        w_e = proj_units(3, 0) + proj_units(3, 1) + [oc_unit(0, 0), oc_unit(0, 1)]
        fin = att_tiles([(0, 2), (1, 2)], weave=w_e, fin_prev=fin)
        w_f = proj_units(3, 2) + proj_units(3, 3) + [oc_unit(0, 2), oc_unit(0, 3)]
        fin = att_tiles([(2, 2), (3, 2)], weave=w_f, fin_prev=fin)
        w_g = [oc_unit(0, oc) for oc in range(4, 10)] + [oc_unit(1, oc) for oc in range(0, 2)]
        fin = att_tiles([(0, 3), (1, 3)], weave=w_g, fin_prev=fin)
        w_h = [oc_unit(0, oc) for oc in range(10, 16)] + [oc_unit(1, oc) for oc in range(2, 4)]
        fin = att_tiles([(2, 3), (3, 3)], weave=w_h, fin_prev=fin)
        fin(tail_weave=[oc_unit(1, oc) for oc in range(4, 8)])
        # dense tail: alternate drain engines so neither DVE nor ACT
        # serializes the chain pipeline
        tail = [(1, oc) for oc in range(8, 16)]
        tail += [(2, oc) for oc in range(NDC)]
        tail += [(3, oc) for oc in range(NDC)]
        for i, (tt, oc) in enumerate(tail):
            outproj_chain(tt, oc, drain_act=i % 2 == 0)

    nc.finalize()
    return nc


def kernel(x, wq, wk, wv, wo, qn_w, kn_w):
    from concourse.bass_utils import run_bass_kernel_spmd

    if "nc" not in _CACHE:
        _CACHE["nc"] = _build_nc()
    nc = _CACHE["nc"]

    x = np.asarray(x, dtype=np.float32)
    wq = np.asarray(wq, dtype=np.float32)
    wk = np.asarray(wk, dtype=np.float32)
    wv = np.asarray(wv, dtype=np.float32)
    wo = np.asarray(wo, dtype=np.float32)
    qn_w = np.asarray(qn_w, dtype=np.float32).reshape(HD, 1).copy()
    kn_w = np.asarray(kn_w, dtype=np.float32).reshape(HD, 1).copy()

    cos2, sin2 = _rope_tables()
    cos2 = cos2.astype(np.float16)
    sin2 = sin2.astype(np.float16)

    in_maps = []
    for c in range(NCORES):
        bc, hc4 = divmod(c, 4)
        xT_c = np.ascontiguousarray(x[bc].T.astype(np.float16))
        wqkv_c = np.ascontiguousarray(
            np.concatenate(
                [
                    wq[:, hc4 * HPC * HD:(hc4 + 1) * HPC * HD],
                    wk[:, hc4 * KPC * HD:(hc4 + 1) * KPC * HD],
                    wv[:, hc4 * KPC * HD:(hc4 + 1) * KPC * HD],
                ],
                axis=1,
            ).astype(np.float16)
        )
        # wo rows for this core's heads -> [128, h*oc*128]
        wo_sl = wo[hc4 * HPC * HD:(hc4 + 1) * HPC * HD, :].astype(np.float16)
        wo_t = np.ascontiguousarray(
            wo_sl.reshape(HPC, 128, NDC, 128).transpose(1, 0, 2, 3).reshape(
                128, HPC * NDC * 128
            )
        )
        in_maps.append(
            {
                "xT": xT_c,
                "wqkv": wqkv_c,
                "woT": wo_t,
                "lcos": cos2,
                "lsin": sin2,
                "qn": qn_w,
                "kn": kn_w,
            }
        )

    trace = bool(_CACHE.get("trace"))
    r = run_bass_kernel_spmd(
        nc, in_maps, core_ids=list(range(NCORES)), trace=trace
    )
    _CACHE["last_result"] = r

    y = np.empty((B, L, D), dtype=np.float32)
    for bc in range(B):
        acc = r.results[bc * 4]["yT"].astype(np.float32)
        for hc4 in range(1, 4):
            acc += r.results[bc * 4 + hc4]["yT"]
        y[bc] = acc.T
    return y


# revision 52
# speedup vs baseline: 1.5575x; 1.0031x over previous
"""Trainium2 Bass kernel for GQA attention (B=2, L=2048, D=2048, H=16, KV=8, HD=128).

Sharding: 2-way data-parallel over batch x 4-way tensor-parallel over heads
(KV-head groups intact). Each core handles one batch and 4 query heads
(2 KV heads): QKV projection + RoPE + RMSNorm + flash-style attention +
a PARTIAL output projection over its 4 heads' rows of wo (row-sharded wo).
The host gather SUMS the 4 partial [D, L] outputs per batch — no on-device
collective. Avoiding collective_compute keeps the PE at full clock.

All heavy matmuls are fp16 with fp32 PSUM accumulation. rstd and softmax
denominators use the Ln/Exp one-table trick and PE ones/broadcast matmuls.

v2 schedule: full-width RoPE ops (host-precomputed [cos;cos] and [-sin;+sin]
tables), paired exp tiles for half-count softmax accumulation adds, and
q-projection half-chains woven into the early attention pairs so the PE
stays fed while the ACT exp stream is the per-pair wall.
"""
import numpy as np

B, L, D = 2, 2048, 2048
H, KV, HD = 16, 8, 128
NCORES = 8
HPC = 4              # query heads per core
KPC = 2              # kv heads per core
EPS = 1e-5
ROPE_BASE = 10000.0
SCALE = HD ** -0.5

TT = 512             # token tile (free dim)
NTT = L // TT        # 4 token tiles
NDC = D // 128       # 16 contraction chunks in qkv proj
NFC = 8              # 4 q + 2 k + 2 v column chunks of 128
QKV_COLS = NFC * 128

_CACHE = {}


def _rope_tables():
    """[128, L] cos2=[cos;cos] and sin2=[-sin;+sin] LUTs, matching the jax
    reference numerics (f32, cpu)."""
    import jax
    import jax.numpy as jnp

    cpu = jax.devices("cpu")[0]
    with jax.default_device(cpu):
        base = ROPE_BASE * 1.0 ** (HD / (HD - 2))
        freqs = base ** (jnp.arange(0, HD, 2, dtype=jnp.float32) / HD)   # [64]
        pos = jnp.arange(L, dtype=jnp.float32)                           # [L]
        angles = pos[:, None] * freqs[None, :]                           # [L, 64]
        cos = np.asarray(jnp.cos(angles), dtype=np.float32).T            # [64, L]
        sin = np.asarray(jnp.sin(angles), dtype=np.float32).T
    cos2 = np.concatenate([cos, cos], axis=0)     # [128, L]
    sin2 = np.concatenate([sin, -sin], axis=0)    # [128, L]
    return np.ascontiguousarray(cos2), np.ascontiguousarray(sin2)


def _build_nc():
    import concourse.bass as bass
    import concourse.tile as tile
    import concourse.mybir as mybir
    from concourse import bacc
    from concourse.masks import make_identity
    from contextlib import ExitStack

    f32 = mybir.dt.float32
    f16 = mybir.dt.float16
    Exp = mybir.ActivationFunctionType.Exp
    Ln = mybir.ActivationFunctionType.Ln
    Copy = mybir.ActivationFunctionType.Copy
    mult = mybir.AluOpType.mult
    add = mybir.AluOpType.add

    from concourse import bacc as _bacc_mod

    if not getattr(_bacc_mod, "_act_table_patch", False):
        _orig_get = _bacc_mod.get_activation_tables

        def _patched_get(arch):
            t = _orig_get(arch)
            exp = mybir.ActivationFunctionType.Exp
            ln = mybir.ActivationFunctionType.Ln
            for name, funcs in t.items():
                if name != "natural_log_exp_and_others":
                    funcs.discard(exp)
                    funcs.discard(ln)
            return t

        _bacc_mod.get_activation_tables = _patched_get
        _bacc_mod._act_table_patch = True

    nc = bacc.Bacc(num_devices=NCORES)

    # per-core inputs (host pre-sliced)
    xT = nc.dram_tensor("xT", [D, L], f16, kind="ExternalInput")
    wqkv = nc.dram_tensor("wqkv", [D, QKV_COLS], f16, kind="ExternalInput")
    # wo rows for this core's 4 heads, tiled [128, h, oc, 128]
    woT = nc.dram_tensor("woT", [128, HPC * NDC * 128], f16, kind="ExternalInput")
    lcos = nc.dram_tensor("lcos", [128, L], f16, kind="ExternalInput")
    lsin = nc.dram_tensor("lsin", [128, L], f16, kind="ExternalInput")
    qn = nc.dram_tensor("qn", [HD, 1], f32, kind="ExternalInput")
    kn = nc.dram_tensor("kn", [HD, 1], f32, kind="ExternalInput")
    # partial output: yT[oc, tok] = sum over this core's heads
    yT = nc.dram_tensor("yT", [D, L], f16, kind="ExternalOutput")

    with tile.TileContext(nc) as tc, ExitStack() as ctx, nc.allow_low_precision(
        reason="f16 storage; all matmul accumulation is fp32 PSUM"
    ):
        consts = ctx.enter_context(tc.tile_pool(name="consts", bufs=1))
        qkvp = ctx.enter_context(tc.tile_pool(name="qkvp", bufs=1))
        ropep = ctx.enter_context(tc.tile_pool(name="ropep", bufs=2))
        statp = ctx.enter_context(tc.tile_pool(name="statp", bufs=2))
        sap = ctx.enter_context(tc.tile_pool(name="sap", bufs=4))
        expp = ctx.enter_context(tc.tile_pool(name="expp", bufs=4))
        attp = ctx.enter_context(tc.tile_pool(name="attp", bufs=2))
        bcp = ctx.enter_context(tc.tile_pool(name="bcp", bufs=2))
        yp = ctx.enter_context(tc.tile_pool(name="yp", bufs=4))

        pbig = ctx.enter_context(tc.tile_pool(name="pbig", bufs=2, space="PSUM"))
        patt = ctx.enter_context(tc.tile_pool(name="patt", bufs=2, space="PSUM"))
        pscore = ctx.enter_context(tc.tile_pool(name="pscore", bufs=4, space="PSUM"))

        # ---- resident tensors. DMA priority order, split small so the
        # 16 queues parallelize and the first wave (kv cols x tt0/tt1)
        # streams right behind the DMA:
        #   kv w-cols + x tt0/tt1 per dc -> x tt2/tt3 -> q w-cols ----
        # Each DMA queue delivers ~15 GB/s (descriptor processing scales with
        # bytes); 16 queues ~ 240 GB/s aggregate. The first wave consumes one
        # dc (8 MMs) per 1.73us, i.e. ~380KB/dc of supply budget — so phase 1
        # carries ONLY w-kv cols + x tt0/tt1 per dc (384KB), partition-halved
        # so per-dc latency is ~8.6us with a ~1.6us/dc pipelined cadence.
        x_sb = consts.tile([128, NDC, L], f16)
        _xr = xT.ap().rearrange("(dc p) t -> p dc t", p=128)
        w_sb = consts.tile([128, NDC, QKV_COLS], f16)
        _wr = wqkv.ap().rearrange("(dc p) f -> p dc f", p=128)

        # dma_start issue costs ~600ns on the issuing sequencer; rotate the
        # issuing engine so 4 queues' worth issue in parallel at startup.
        _dma_engines = [nc.sync, nc.gpsimd, nc.scalar]
        _di = [0]

        def dma(out, in_):
            e = _dma_engines[_di[0] % 3]
            _di[0] += 1
            e.dma_start(out=out, in_=in_)

        for dc in range(NDC):
            nsp = 4 if dc < 2 else 2
            for s in range(nsp):
                p0, p1 = s * 128 // nsp, (s + 1) * 128 // nsp
                dma(w_sb[p0:p1, dc:dc + 1, 512:1024],
                    _wr[p0:p1, dc:dc + 1, 512:1024])
            for s in range(nsp):
                p0, p1 = s * 128 // nsp, (s + 1) * 128 // nsp
                dma(x_sb[p0:p1, dc:dc + 1, 0:2 * TT],
                    _xr[p0:p1, dc:dc + 1, 0:2 * TT])
        # phase 2: x tt2/tt3 (wave2 feed), then q weight cols (proj_tt(0) feed)
        for dc in range(NDC):
            for s in range(2):
                p0, p1 = s * 64, (s + 1) * 64
                dma(x_sb[p0:p1, dc:dc + 1, 2 * TT:L],
                    _xr[p0:p1, dc:dc + 1, 2 * TT:L])
        for dc in range(NDC):
            for s in range(2):
                p0, p1 = s * 64, (s + 1) * 64
                dma(w_sb[p0:p1, dc:dc + 1, 0:512],
                    _wr[p0:p1, dc:dc + 1, 0:512])
        # ---- constants ----
        ones_f = consts.tile([128, 1], f32)
        nc.vector.memset(ones_f, 1.0)
        ones = consts.tile([128, 1], f16)
        nc.vector.tensor_copy(out=ones, in_=ones_f)
        ident = consts.tile([128, 128], f16)
        make_identity(nc, ident)
        eps_t = consts.tile([1, 1], f32)
        nc.vector.memset(eps_t, EPS)
        neg8 = consts.tile([128, 1], f32)
        nc.vector.memset(neg8, -8.0)
        cos_sb = consts.tile([128, L], f16)
        nc.gpsimd.dma_start(out=cos_sb, in_=lcos[:, :])
        sin_sb = consts.tile([128, L], f16)
        nc.gpsimd.dma_start(out=sin_sb, in_=lsin[:, :])
        qn_sb = consts.tile([HD, 1], f32)
        nc.gpsimd.dma_start(out=qn_sb, in_=qn[:, :])
        kn_sb = consts.tile([HD, 1], f32)
        nc.gpsimd.dma_start(out=kn_sb, in_=kn[:, :])
        # wo tiles [128, h, oc, 128] (needed only at out-projection)
        wo_sb = consts.tile([128, HPC, NDC, 128], f16)
        _wor = woT.ap().rearrange("p (h oc m) -> p h oc m", h=HPC, oc=NDC)
        nc.sync.dma_start(out=wo_sb[:, 0:2, :, :], in_=_wor[:, 0:2, :, :])
        nc.sync.dma_start(out=wo_sb[:, 2:4, :, :], in_=_wor[:, 2:4, :, :])

        # activations
        qh_t = [qkvp.tile([128, L], f16, name=f"qh{h}") for h in range(HPC)]
        kh_t = [qkvp.tile([128, L], f16, name=f"kh{g}") for g in range(KPC)]
        v_t = [qkvp.tile([128, L // 128, HD], f16, name=f"v{g}") for g in range(KPC)]
        a_t = [qkvp.tile([128, L], f16, name=f"a{h}") for h in range(HPC)]

        def proj_post(tt, fc, rsrc, act_swap=False, pss=None):
            """RoPE + RMSNorm (q/k) or transpose (v) for a staged projection tile.

            RoPE with full-width ops: cos_sb = [cos;cos], sin_sb = [-sin;+sin];
            roped = rsrc*cos2 + swap(rsrc)*sin2."""
            pos0 = tt * TT
            if fc < 6:
                # RoPE preserves per-token sum of squares, so the rstd chain
                # runs off the pre-rope tile, in parallel with the rotation.
                sq = ropep.tile([128, TT], f16, tag="tb", name="sq")
                nc.vector.tensor_tensor(out=sq, in0=rsrc, in1=rsrc, op=mult)
                if pss is None:
                    pt = pscore.tile([1, TT], f32, tag="ps", name="pt")
                else:
                    pt = pscore.tile([1, TT], f32, tag="ps", name="ptsm")
                nc.tensor.matmul(pt, ones, sq, start=True, stop=True)
                lsrc = pt
                lnt = statp.tile([1, TT], f32, tag="statf")
                nc.scalar.activation(
                    out=lnt, in_=lsrc, func=Ln, bias=eps_t, scale=1.0 / HD
                )
                srd = statp.tile([1, TT], f16, tag="stath")
                nc.scalar.activation(out=srd, in_=lnt, func=Exp, scale=-0.5)
                pb = bcp.tile([128, TT], f16, tag="bc")
                nc.gpsimd.partition_broadcast(out_ap=pb, in_ap=srd)
                # rotation: ta = rsrc*[cos;cos]; tbs = rsrc*[+sin;-sin];
                # tb2 = swap(tbs) halves; roped = ta + tb2 (in place).
                ta = ropep.tile([128, TT], f16, tag="ta")
                nc.vector.tensor_tensor(
                    out=ta, in0=rsrc, in1=cos_sb[:, pos0:pos0 + TT], op=mult
                )
                tbs = ropep.tile([128, TT], f16, tag="tb", name="tbs")
                nc.vector.tensor_tensor(
                    out=tbs, in0=rsrc, in1=sin_sb[:, pos0:pos0 + TT], op=mult
                )
                tb2 = ropep.tile([128, TT], f16, tag="tb2")
                if act_swap:
                    # offload the half swaps to ACT in phases where it idles
                    nc.scalar.activation(
                        out=tb2[0:64, :], in_=tbs[64:128, :], func=Copy
                    )
                    nc.scalar.activation(
                        out=tb2[64:128, :], in_=tbs[0:64, :], func=Copy
                    )
                else:
                    nc.vector.tensor_copy(out=tb2[0:64, :], in_=tbs[64:128, :])
                    nc.vector.tensor_copy(out=tb2[64:128, :], in_=tbs[0:64, :])
                roped = ta
                nc.vector.tensor_tensor(out=roped, in0=ta, in1=tb2, op=add)
                w_head = qn_sb if fc < 4 else kn_sb
                if fc < 4:
                    dst = qh_t[fc][:, pos0:pos0 + TT]
                else:
                    dst = kh_t[fc - 4][:, pos0:pos0 + TT]
                nc.vector.scalar_tensor_tensor(
                    out=dst, in0=roped, scalar=w_head, in1=pb,
                    op0=mult, op1=mult,
                )
            else:
                g = fc - 6
                for i in range(TT // 128):
                    pt = pscore.tile([128, 128], f16, tag="ps")
                    nc.tensor.transpose(pt, rsrc[:, i * 128:(i + 1) * 128], ident)
                    nc.vector.tensor_copy(out=v_t[g][:, tt * 4 + i, :], in_=pt)

        def proj_chain(tt, fc):
            """Emit the 16-chunk projection chain for (tt, fc); return staged copy."""
            pos0 = tt * TT
            pp = pbig.tile([128, TT], f32, tag="pp", name=f"pp{fc}")
            for dc in range(NDC):
                nc.tensor.matmul(
                    pp,
                    w_sb[:, dc, fc * 128:(fc + 1) * 128],
                    x_sb[:, dc, pos0:pos0 + TT],
                    start=(dc == 0),
                    stop=(dc == NDC - 1),
                )
            rsrc = ropep.tile([128, TT], f16, tag="rsrc", bufs=4)
            nc.vector.tensor_copy(out=rsrc, in_=pp)
            return rsrc

        def proj_units(tt, fc):
            """Split one (tt, fc) projection chain into weave units:
            [8 MMs] [8 MMs + drain] [post]."""
            pos0 = tt * TT
            st = {}

            def ua():
                st["pp"] = pbig.tile([128, TT], f32, tag="pp", name=f"pw{fc}{tt}")
                for dc in range(8):
                    nc.tensor.matmul(
                        st["pp"],
                        w_sb[:, dc, fc * 128:(fc + 1) * 128],
                        x_sb[:, dc, pos0:pos0 + TT],
                        start=(dc == 0),
                        stop=False,
                    )

            def ub():
                for dc in range(8, NDC):
                    nc.tensor.matmul(
                        st["pp"],
                        w_sb[:, dc, fc * 128:(fc + 1) * 128],
                        x_sb[:, dc, pos0:pos0 + TT],
                        start=False,
                        stop=(dc == NDC - 1),
                    )
                rsrc = ropep.tile([128, TT], f16, tag="rsrc", bufs=4)
                nc.vector.tensor_copy(out=rsrc, in_=st["pp"])
                st["rsrc"] = rsrc

            def uc():
                # ssq via psmall: don't steal a score bank mid-attention
                proj_post(tt, fc, st["rsrc"])

            return [ua, ub, uc]

        def proj_wave(pairs, stage_dsts):
            """8 projection chains at once (one PSUM bank each, borrowed across
            pools) so the PE keeps pace with the initial per-dc x DMA stream.

            Results are staged into stage_dsts (idle a_t slices — unused until
            the attention finalizers, long after the posts consume them), so
            no staging-ring pressure. Returns deferred post closures."""
            slots = [
                pbig.tile([128, TT], f32, tag="pp", name="wv0"),
                pbig.tile([128, TT], f32, tag="pp", name="wv1"),
                patt.tile([128, TT], f32, tag="po", name="wv2"),
                patt.tile([128, TT], f32, tag="po", name="wv3"),
                pscore.tile([128, TT], f32, tag="ps", name="wv4"),
                pscore.tile([128, TT], f32, tag="ps", name="wv5"),
                pscore.tile([128, TT], f32, tag="ps", name="wv6"),
                pscore.tile([128, TT], f32, tag="ps", name="wv7"),
            ]
            assert len(pairs) == 8
            for dc in range(NDC):
                for i, (fc, tt) in enumerate(pairs):
                    pos0 = tt * TT
                    nc.tensor.matmul(
                        slots[i],
                        w_sb[:, dc, fc * 128:(fc + 1) * 128],
                        x_sb[:, dc, pos0:pos0 + TT],
                        start=(dc == 0),
                        stop=(dc == NDC - 1),
                    )
            staged = []
            for i, (fc, tt) in enumerate(pairs):
                dst = stage_dsts[i]
                nc.vector.tensor_copy(out=dst, in_=slots[i])
                staged.append((tt, fc, dst))
            return staged

        def proj_tt(tt, fcs, sprinkle=()):
            """Projection for one token tile: chains run two fc ahead of
            postprocessing; deferred wave posts sprinkle between chains."""
            sprinkle = list(sprinkle)
            pend = []
            for fc in fcs:
                rsrc = proj_chain(tt, fc)
                if sprinkle:
                    sprinkle.pop(0)()
                if sprinkle:
                    sprinkle.pop(0)()
                pend.append((fc, rsrc))
                if len(pend) > 1:
                    wfc, wrs = pend.pop(0)
                    proj_post(tt, wfc, wrs)
            for u in sprinkle:
                u()
            for wfc, wrs in pend:
                proj_post(tt, wfc, wrs)

        def outproj_chain(tt, oc, drain_act=False):
            """One 128-col partial out-projection chain + drain."""
            pos0 = tt * TT
            py = pbig.tile([128, TT], f32, tag="pp", name=f"py{oc % 2}")
            for h in range(HPC):
                nc.tensor.matmul(
                    py, wo_sb[:, h, oc, :], a_t[h][:, pos0:pos0 + TT],
                    start=(h == 0), stop=(h == HPC - 1),
                )
            yt = yp.tile([128, TT], f16, tag="y")
            if drain_act:
                nc.scalar.activation(out=yt, in_=py, func=Copy, scale=1.0)
            else:
                nc.vector.tensor_copy(out=yt, in_=py)
            nc.sync.dma_start(
                out=yT[oc * 128:(oc + 1) * 128, pos0:pos0 + TT], in_=yt
            )

        def oc_unit(tt, oc, drain_act=False):
            return lambda: outproj_chain(tt, oc, drain_act=drain_act)

        def att_tiles(jobs, weave=(), fin_prev=None):
            """Interleaved attention for (h, tqt) query tiles (same kv group).

            weave: list of closures (out-projection chains / q-proj units)
            emitted between key chunks to keep the PE fed while the ACT exp
            stream drains. fin_prev: finish closure of the previous pair,
            emitted after the first key chunks so its DVE-latency-bound
            normalize chain hides under this pair's score stream. Returns
            this pair's finish closure.

            exp tiles are paired [128, 2, TT] so the softmax running-sum adds
            run at [128, 2*TT] width, halving the DVE instruction count."""
            NK = L // 128
            weave = list(weave)
            state = []
            for h, tqt in jobs:
                g = h // 2
                qs = qh_t[h][:, tqt * TT:(tqt + 1) * TT]
                po = patt.tile([128, TT], f32, tag="po", name=f"po{h}{tqt}")
                sacc = sap.tile([128, 2, TT], f16, tag="sacc", bufs=3,
                                name=f"sa{h}{tqt}")
                state.append((h, g, tqt, qs, po, sacc, {}))
            nweave = len(weave)
            for tk in range(NK):
                for h, g, tqt, qs, po, sacc, st in state:
                    ps = pscore.tile([128, TT], f32, tag="ps")
                    nc.tensor.matmul(
                        ps, kh_t[g][:, tk * 128:(tk + 1) * 128], qs,
                        start=True, stop=True,
                    )
                    if tk % 2 == 0:
                        st["et2"] = expp.tile(
                            [128, 2, TT], f16, tag="expt", name=f"et{h}{tk}"
                        )
                    et = st["et2"][:, tk % 2, :]
                    # -8 exponent shift (softmax-invariant) keeps the po/v
                    # accumulator magnitudes small enough for f16 staging
                    nc.scalar.activation(
                        out=et, in_=ps, func=Exp, bias=neg8, scale=SCALE
                    )
                if tk == 0 and fin_prev is not None:
                    fin_prev()
                    fin_prev = None
                for h, g, tqt, qs, po, sacc, st in state:
                    et = st["et2"][:, tk % 2, :]
                    nc.tensor.matmul(
                        po, v_t[g][:, tk, :], et,
                        start=(tk == 0), stop=(tk == NK - 1),
                    )
                    if tk % 2 == 1:
                        pair = st["et2"][:, :, :]
                        sview = sacc[:, :, :]
                        if tk == 1:
                            nc.vector.tensor_copy(out=sview, in_=pair)
                        else:
                            nc.vector.tensor_tensor(
                                out=sview, in0=sview, in1=pair, op=add
                            )
                # weave filler chains uniformly across the key loop
                while weave and len(weave) > nweave * (NK - 1 - tk) // NK:
                    weave.pop(0)()

            def fin(tail_weave=()):
                pds = []
                for h, g, tqt, qs, po, sacc, st in state:
                    pd = pscore.tile([1, TT], f32, tag="ps", name="pd")
                    nc.tensor.matmul(
                        pd, ones, sacc[:, 0, :], start=True, stop=False
                    )
                    nc.tensor.matmul(
                        pd, ones, sacc[:, 1, :], start=False, stop=True
                    )
                    rdf = statp.tile([1, TT], f32, tag="statf")
                    nc.vector.reciprocal_approx_fast(out=rdf, in_=pd)
                    pds.append(rdf)
                for u in tail_weave:
                    u()
                # Batches of 2: po-releasing copies and broadcast issues first,
                # broadcast-dependent multiplies after, so the in-order DVE
                # queue doesn't stall on a GpSimd broadcast before freeing po.
                pairs2 = [(s[0], s[2], s[4], rdf) for s, rdf in zip(state, pds)]
                for j0 in range(0, len(pairs2), 2):
                    staged2 = []
                    for h, tqt, po, rdf in pairs2[j0:j0 + 2]:
                        rd = statp.tile([1, TT], f16, tag="stath")
                        nc.vector.tensor_copy(out=rd, in_=rdf)
                        pb = bcp.tile([128, TT], f16, tag="bc")
                        nc.gpsimd.partition_broadcast(out_ap=pb, in_ap=rd)
                        o_sb = attp.tile([128, TT], f16, tag="att")
                        nc.vector.tensor_copy(out=o_sb, in_=po)
                        staged2.append((h, tqt, pb, o_sb))
                    for h, tqt, pb, o_sb in staged2:
                        nc.vector.tensor_tensor(
                            out=a_t[h][:, tqt * TT:(tqt + 1) * TT],
                            in0=o_sb, in1=pb, op=mult,
                        )

            return fin

        def outproj_tt(tt, drain_act=False):
            for oc in range(NDC):
                outproj_chain(tt, oc, drain_act=drain_act and oc % 2 == 0)

        # ---- schedule ----
        # KV proj waves first (posts deferred into the next MM stream);
        # q proj tt0 dense; q proj tt1-3 woven into the early attention pairs
        # as half-chain units; the out-projection woven into later pairs so
        # the PE never starves while the ACT exp stream drains.
        def wave_dsts(a_lo, a_hi):
            return [a_t[a_lo][:, s * TT:(s + 1) * TT] for s in range(4)] + [
                a_t[a_hi][:, s * TT:(s + 1) * TT] for s in range(4)
            ]

        st1 = proj_wave(
            [(4, 0), (5, 0), (6, 0), (7, 0), (4, 1), (5, 1), (6, 1), (7, 1)],
            stage_dsts=wave_dsts(0, 1),
        )
        st2 = proj_wave(
            [(4, 2), (5, 2), (6, 2), (7, 2), (4, 3), (5, 3), (6, 3), (7, 3)],
            stage_dsts=wave_dsts(2, 3),
        )

        def wpost(item, **kw):
            tt, fc, r = item
            proj_post(tt, fc, r, **kw)

        # wave1 posts, v-transposes first (their data staged long before so
        # the PE never waits on fresh DVE); k-posts share a packed ssq tile
        for i in (2, 3, 6, 7):
            wpost(st1[i])
        for i in (0, 1, 4, 5):
            wpost(st1[i], act_swap=True)
        # proj_tt(0) by hand: wave2's v-posts + 2 k-posts sprinkle between
        # chains; the (4,3)/(5,3) k-posts and the q2/q3 posts defer into
        # pair A's weave (ACT slack there; their consumers come later).
        r0 = proj_chain(0, 0)
        wpost(st2[2])                     # v (6,2)
        wpost(st2[3])                     # v (7,2)
        r1 = proj_chain(0, 1)
        wpost(st2[6])                     # v (6,3)
        wpost(st2[7])                     # v (7,3)
        proj_post(0, 0, r0, act_swap=True)
        r2 = proj_chain(0, 2)
        wpost(st2[0], act_swap=True)   # k (4,2)
        wpost(st2[1], act_swap=True)   # k (5,2)
        proj_post(0, 1, r1, act_swap=True)
        r3 = proj_chain(0, 3)

        wpost(st2[4], act_swap=True)      # k (4,3)
        proj_post(0, 2, r2, act_swap=True)
        wpost(st2[5], act_swap=True)      # k (5,3)
        proj_post(0, 3, r3, act_swap=True)

        w_a = proj_units(1, 0) + proj_units(1, 1)
        fin = att_tiles([(0, 0), (1, 0)], weave=w_a)
        w_b = proj_units(1, 2) + proj_units(1, 3)
        fin = att_tiles([(2, 0), (3, 0)], weave=w_b, fin_prev=fin)
        w_c = proj_units(2, 0) + proj_units(2, 1)
        fin = att_tiles([(0, 1), (1, 1)], weave=w_c, fin_prev=fin)
        w_d = proj_units(2, 2) + proj_units(2, 3)
        fin = att_tiles([(2, 1), (3, 1)], weave=w_d, fin_prev=fin)
        w_e = proj_units(3, 0) + proj_units(3, 1) + [oc_unit(0, 0), oc_unit(0, 1)]
        fin = att_tiles([(0, 2), (1, 2)], weave=w_e, fin_prev=fin)
        w_f = proj_units(3, 2) + proj_units(3, 3) + [oc_unit(0, 2), oc_unit(0, 3)]
        fin = att_tiles([(2, 2), (3, 2)], weave=w_f, fin_prev=fin)
        w_g = [oc_unit(0, oc) for oc in range(4, 10)] + [oc_unit(1, oc) for oc in range(0, 2)]
        fin = att_tiles([(0, 3), (1, 3)], weave=w_g, fin_prev=fin)
        w_h = [oc_unit(0, oc) for oc in range(10, 16)] + [oc_unit(1, oc) for oc in range(2, 4)]
        fin = att_tiles([(2, 3), (3, 3)], weave=w_h, fin_prev=fin)
        fin(tail_weave=[oc_unit(1, oc) for oc in range(4, 8)])
        # dense tail: alternate drain engines so neither DVE nor ACT
        # serializes the chain pipeline
        tail = [(1, oc) for oc in range(8, 16)]
        tail += [(2, oc) for oc in range(NDC)]
        tail += [(3, oc) for oc in range(NDC)]
        for i, (tt, oc) in enumerate(tail):
            outproj_chain(tt, oc, drain_act=i % 2 == 0)

    nc.finalize()
    return nc


def kernel(x, wq, wk, wv, wo, qn_w, kn_w):
    from concourse.bass_utils import run_bass_kernel_spmd

    if "nc" not in _CACHE:
        _CACHE["nc"] = _build_nc()
    nc = _CACHE["nc"]

    x = np.asarray(x, dtype=np.float32)
    wq = np.asarray(wq, dtype=np.float32)
    wk = np.asarray(wk, dtype=np.float32)
    wv = np.asarray(wv, dtype=np.float32)
    wo = np.asarray(wo, dtype=np.float32)
    qn_w = np.asarray(qn_w, dtype=np.float32).reshape(HD, 1).copy()
    kn_w = np.asarray(kn_w, dtype=np.float32).reshape(HD, 1).copy()

    cos2, sin2 = _rope_tables()
    cos2 = cos2.astype(np.float16)
    sin2 = sin2.astype(np.float16)

    in_maps = []
    for c in range(NCORES):
        bc, hc4 = divmod(c, 4)
        xT_c = np.ascontiguousarray(x[bc].T.astype(np.float16))
        wqkv_c = np.ascontiguousarray(
            np.concatenate(
                [
                    wq[:, hc4 * HPC * HD:(hc4 + 1) * HPC * HD],
                    wk[:, hc4 * KPC * HD:(hc4 + 1) * KPC * HD],
                    wv[:, hc4 * KPC * HD:(hc4 + 1) * KPC * HD],
                ],
                axis=1,
            ).astype(np.float16)
        )
        # wo rows for this core's heads -> [128, h*oc*128]
        wo_sl = wo[hc4 * HPC * HD:(hc4 + 1) * HPC * HD, :].astype(np.float16)
        wo_t = np.ascontiguousarray(
            wo_sl.reshape(HPC, 128, NDC, 128).transpose(1, 0, 2, 3).reshape(
                128, HPC * NDC * 128
            )
        )
        in_maps.append(
            {
                "xT": xT_c,
                "wqkv": wqkv_c,
                "woT": wo_t,
                "lcos": cos2,
                "lsin": sin2,
                "qn": qn_w,
                "kn": kn_w,
            }
        )

    trace = bool(_CACHE.get("trace"))
    r = run_bass_kernel_spmd(
        nc, in_maps, core_ids=list(range(NCORES)), trace=trace
    )
    _CACHE["last_result"] = r

    y = np.empty((B, L, D), dtype=np.float32)
    for bc in range(B):
        acc = r.results[bc * 4]["yT"].astype(np.float32)
        for hc4 in range(1, 4):
            acc += r.results[bc * 4 + hc4]["yT"]
        y[bc] = acc.T
    return y
